# revision 9
# baseline (speedup 1.0000x reference)
"""2-layer GCN (GCNConv x2) on 8 Trainium2 NeuronCores.

Strategy (dst-sharded, edge-partitioned by destination; Q7-descgen-optimized):
- Each core owns N/8 destination nodes and the edges pointing at them
  (plus the GCN self-loops, kept in the edge stream).
- Table layout: 4 banks, bank q = concat over cores of quarter-q of their
  shard (block-aligned quarters, int16-indexable). Layer 1's table
  h~ = dinv * (x @ W1) is computed REPLICATED on every core (x is a
  shared input; ~83us of idle PE + sequential DMA), written straight
  into the bank tensors in bank-major order so bank q is ready at
  ~(q+1)/4 of the h1 phase -- no AllGather and no collective-serialization
  in the critical path.
- Per (super-block of 6 dst blocks, bank): edges packed contiguously in
  block order into 128-slot chunks (straddling block boundaries); one
  dma_gather per segment on queue=bank (4 SWDGE queue contexts, balanced).
  Scatter-add via is_equal-indicator matmuls accumulating in PSUM.
- idx tiles and epilogue dv tiles are prefetched one super-block ahead so
  the sync-engine FIFO (epilogue stores) never blocks gather dispatch.
- Layer 1 accumulates transposed (aggT [feat, dst]) so bias+ReLU ride the
  activation engine and out1 feeds h2 = out1 @ W2 directly as lhsT;
  h2~ = dinv * h2 goes out bf16 through 4 bank-wise AllGathers placed
  as their quarters complete, so layer 2's gathers pipeline behind
  layer 1 and only the last bank pays a collective latency.
"""
import sys
import types

import numpy as np
import ml_dtypes

P = 128
NCORES = 8
GMAX = 32  # max chunks (128 idxs each) per dma_gather
SB_N = 6  # dst blocks per super-block (one PSUM bank each; 6+1+1 banks)
NQUEUES = 4
XGRP = 8
GBUFS = 9
SBUFS = 6
AG2_LAG = 2  # super-blocks between a quarter finishing and its AG2 trigger

_CACHE = {}


# ---------------------------------------------------------------- compat ---
def _install_compat():
    """Patches for this axon/walrus stack (drain waits, per-inst wait caps,
    NTFF shim). Idempotent."""
    if _CACHE.get("compat"):
        return
    import concourse.tile as tile
    import concourse.mybir as mybir

    _ev = [0]

    def _split_inst_waits(ordered):
        for _bb, insts in ordered.items():
            out = []
            for inst in insts:
                si = getattr(inst, "sync_info", None)
                if si is not None and si.on_wait is not None and len(si.on_wait) > 1:
                    waits = list(si.on_wait)
                    excess, keep = waits[:-1], waits[-1:]
                    si.on_wait.clear()
                    for sw in keep:
                        si.on_wait.append(sw)
                    for i in range(0, len(excess), 2):
                        _ev[0] += 1
                        ev = mybir.InstEventSemaphore(
                            name=f"evsplit-{_ev[0]}", ins=[], outs=[]
                        )
                        ev.engine = inst.engine
                        ev.sync_info = mybir.SyncInfo(
                            on_wait=excess[i : i + 2], on_update=[]
                        )
                        out.append(ev)
                out.append(inst)
            insts[:] = out

    orig_lower = tile.TileContext._lower_ordered_insts

    def patched_lower(self, ordered):
        _split_inst_waits(ordered)
        return orig_lower(self, ordered)

    def patched_drain(self, tick_clock, wait_clock):
        sems_alloc = list(self.sems.allocated().values())
        carrier = self.nc.sync.wait_ge(sems_alloc[0], 0)
        wait_clock.add_sem_waits(
            carrier.ins, tile.ScopedClock({None: tick_clock.global_clock})
        )
        waits = list(carrier.ins.sync_info.on_wait)
        carrier.ins.sync_info.on_wait.clear()
        for sw in waits[:2]:
            carrier.ins.sync_info.on_wait.append(sw)
        for i in range(2, len(waits), 2):
            c = self.nc.sync.wait_ge(sems_alloc[0], 0)
            c.ins.sync_info.on_wait.clear()
            for sw in waits[i : i + 2]:
                c.ins.sync_info.on_wait.append(sw)
        self.nc.sync.drain(fusable=False)
        self.nc.all_engine_barrier()
        popped = self.nc._tile_sem_poison_stack.pop()
        assert popped is self._sem_poison
        self.nc.clear_and_free_semaphores(sems_alloc)
        self.nc.all_engine_barrier()

    tile.TileContext._lower_ordered_insts = patched_lower
    tile.TileContext._drain_and_barrier = patched_drain

    # NTFF profile hook shim (missing antenv.axon_hooks in this image)
    _hook = {}
    mod = types.ModuleType("antenv.axon_hooks")
    mod.set_axon_ntff_profile_hook = lambda h: _hook.update(hook=h)
    mod.get_axon_ntff_profile_hook = lambda: _hook.get("hook")
    sys.modules["antenv.axon_hooks"] = mod
    try:
        import antenv

        antenv.axon_hooks = mod
        from trn_agent_boot.trn_boot import _ntff_profile_via_ctypes

        mod.set_axon_ntff_profile_hook(
            _ntff_profile_via_ctypes("/opt/axon/libaxon_pjrt.so")
        )
    except Exception:
        pass
    _CACHE["compat"] = True


# ---------------------------------------------------------- preprocessing ---
class Schedule:
    pass


def _quarter_bounds(nblk):
    base, rem = nblk // 4, nblk % 4
    sizes = [base + (1 if i < rem else 0) for i in range(4)]
    starts = np.cumsum([0] + sizes)
    return [(int(starts[i]), int(starts[i + 1])) for i in range(4)]


def _preprocess(n, edge_index):
    src0 = np.asarray(edge_index[0], np.int64)
    dst0 = np.asarray(edge_index[1], np.int64)
    shard = n // NCORES
    nblk = (shard + P - 1) // P
    n_sb = (nblk + SB_N - 1) // SB_N
    qb = _quarter_bounds(nblk)
    qrow_start = [b0 * P for b0, b1 in qb]
    qrows = [min(b1 * P, shard) - b0 * P for b0, b1 in qb]
    bank_n = [NCORES * r for r in qrows]
    assert all(b <= 32767 for b in bank_n)

    deg = np.bincount(dst0, minlength=n).astype(np.float64) + 1.0
    dinv = (1.0 / np.sqrt(deg)).astype(np.float32)

    # append GCN self-loops as regular edges
    loops = np.arange(n, dtype=np.int64)
    src = np.concatenate([src0, loops])
    dst = np.concatenate([dst0, loops])
    e = src.shape[0]

    core_d = dst // shard
    dl = dst - core_d * shard
    blk = dl // P
    dstloc = (dl % P).astype(np.int64)
    sb = blk // SB_N
    core_s = src // shard
    off = src - core_s * shard
    sblk = off // P
    qb_arr = np.zeros(nblk, np.int64)
    for q, (b0, b1) in enumerate(qb):
        qb_arr[b0:b1] = q
    bank = qb_arr[sblk]
    bidx = (
        core_s * np.array(qrows)[bank] + (off - np.array(qrow_start)[bank])
    ).astype(np.int64)

    order = np.lexsort((blk, bank, sb, core_d))
    s_core = core_d[order]
    s_sb = sb[order]
    s_bank = bank[order]
    s_blk = blk[order]
    s_bidx = bidx[order]
    s_dstloc = dstloc[order]

    key = ((s_core * n_sb + s_sb) * 4 + s_bank) * nblk + s_blk
    cnt = np.bincount(key, minlength=NCORES * n_sb * 4 * nblk).reshape(
        NCORES, n_sb, 4, nblk
    )

    gathers = []
    slots = []
    first_slot_of_block = {}
    last_slot_of_block = {}
    chunk_gid = 0
    slot_gid = 0
    budget_tab = np.zeros((n_sb, 4), np.int64)
    for s in range(n_sb):
        blocks = list(range(s * SB_N, min((s + 1) * SB_N, nblk)))
        for k in range(4):
            percore = cnt[:, s, k, :][:, blocks]
            cum = np.cumsum(percore, axis=1)
            budget = max(int(np.ceil(cum[:, -1].max() / P)), 1)
            budget_tab[s, k] = budget
            lo = np.min(cum - percore, axis=0)
            hi = np.max(cum, axis=0)
            g0 = 0
            while g0 < budget:
                gn = min(GMAX, budget - g0)
                gi = len(gathers)
                gslot0 = slot_gid
                for j in range(g0, g0 + gn):
                    c_lo, c_hi = j * P, (j + 1) * P
                    for bi, b in enumerate(blocks):
                        if hi[bi] > c_lo and lo[bi] < c_hi:
                            slots.append(
                                dict(
                                    g=gi,
                                    cl=j - g0,
                                    blk=b,
                                    sb=s,
                                    bank=k,
                                    sl_local=slot_gid - gslot0,
                                    chunk_gid=chunk_gid + j,
                                )
                            )
                            first_slot_of_block.setdefault((s, b), slot_gid)
                            last_slot_of_block[(s, b)] = slot_gid
                            slot_gid += 1
                gathers.append(
                    dict(
                        gi=gi,
                        sb=s,
                        bank=k,
                        chunk0=chunk_gid + g0,
                        nch=gn,
                        nidx=gn * P,
                        slot0=gslot0,
                        nslots=slot_gid - gslot0,
                        c16=(chunk_gid + g0) * P // 16,
                    )
                )
                g0 += gn
            chunk_gid += budget
    totc = chunk_gid
    nslots = slot_gid
    tot_slots = totc * P
    for i, sl in enumerate(slots):
        sl["start"] = first_slot_of_block[(sl["sb"], sl["blk"])] == i
        sl["stop"] = last_slot_of_block[(sl["sb"], sl["blk"])] == i
    slotmax = max(g["nslots"] for g in gathers)
    gchmax = max(g["nch"] for g in gathers)

    seg_key = (s_core * n_sb + s_sb) * 4 + s_bank
    seg_ptr = np.searchsorted(seg_key, np.arange(NCORES * n_sb * 4 + 1))
    seg_chunk0 = {}
    cg = 0
    for s in range(n_sb):
        for k in range(4):
            seg_chunk0[(s, k)] = cg
            cg += int(budget_tab[s, k])

    idx_flat = np.zeros((NCORES, tot_slots), np.int16)
    dstloc_s = np.full((NCORES, P, nslots), -1.0, np.float32)
    for c in range(NCORES):
        arr = idx_flat[c]
        for s in range(n_sb):
            for k in range(4):
                p0 = seg_ptr[(c * n_sb + s) * 4 + k]
                p1 = seg_ptr[(c * n_sb + s) * 4 + k + 1]
                base = seg_chunk0[(s, k)] * P
                arr[base : base + (p1 - p0)] = s_bidx[p0:p1].astype(np.int16)
                # pads stay 0 (gather row 0; indicator -1 masks them out).

    seg_id = (s_core * n_sb + s_sb) * 4 + s_bank
    pos_in_seg = np.arange(e) - seg_ptr[seg_id]
    seg_chunk0_arr = np.zeros(NCORES * n_sb * 4, np.int64)
    for s in range(n_sb):
        for k in range(4):
            for c in range(NCORES):
                seg_chunk0_arr[(c * n_sb + s) * 4 + k] = seg_chunk0[(s, k)]
    chunk_of_edge = seg_chunk0_arr[seg_id] + pos_in_seg // P
    part_of_edge = pos_in_seg % P
    slot_lut = np.full((totc, nblk), -1, np.int64)
    for i, sl in enumerate(slots):
        slot_lut[sl["chunk_gid"], sl["blk"]] = i
    slot_of_edge = slot_lut[chunk_of_edge, s_blk]
    assert (slot_of_edge >= 0).all()
    dstloc_s[s_core, part_of_edge, slot_of_edge] = s_dstloc

    # wrap idx stream: slot i -> [lane i%16, col i//16], replicate to 128 parts
    idx_stream = np.ascontiguousarray(
        idx_flat.reshape(NCORES, tot_slots // 16, 16).transpose(0, 2, 1)
    )
    idx_stream = np.tile(idx_stream, (1, 8, 1))

    # table node order (bank-major) for the replicated h1 phase
    nodes = []
    for q in range(4):
        idx = (
            np.arange(NCORES)[:, None] * shard
            + qrow_start[q]
            + np.arange(qrows[q])[None, :]
        ).ravel()
        nodes.append(idx)
    table_nodes = np.concatenate(nodes)
    assert table_nodes.shape[0] == n
    tblk_per_bank = [-(-b // P) for b in bank_n]
    tblk_total = sum(tblk_per_bank)

    sch = Schedule()
    sch.n, sch.e, sch.shard, sch.nblk, sch.n_sb = n, e, shard, nblk, n_sb
    sch.qb, sch.qrow_start, sch.qrows, sch.bank_n = qb, qrow_start, qrows, bank_n
    sch.dinv = dinv
    sch.gathers = gathers
    sch.slots = slots
    sch.totc, sch.nslots, sch.tot_slots = totc, nslots, tot_slots
    sch.slotmax = slotmax
    sch.gchmax = gchmax
    sch.idx_stream = idx_stream
    sch.idx_flat = idx_flat
    sch.dstloc_s = dstloc_s.astype(ml_dtypes.bfloat16)
    sch.table_nodes = table_nodes
    sch.tblk_per_bank = tblk_per_bank
    sch.tblk_total = tblk_total
    return sch


# ----------------------------------------------------------------- build ---
def _build(sch, in_dim, hid, out_dim):
    import concourse.mybir as mybir
    import concourse.tile as tile
    from concourse import bacc

    bf16 = mybir.dt.bfloat16
    f32 = mybir.dt.float32
    shard, nblk, n_sb = sch.shard, sch.nblk, sch.n_sb
    slotmax, gchmax = sch.slotmax, sch.gchmax
    qb = sch.qb
    n = sch.n

    nc = bacc.Bacc(num_swdge_queues=NQUEUES)

    xT = nc.declare_dram_parameter("xT", [in_dim, n], bf16, isOutput=False)
    idxs = nc.declare_dram_parameter(
        "idxs", [P, sch.tot_slots // 16], mybir.dt.int16, isOutput=False
    )
    dstloc = nc.declare_dram_parameter("dstloc", [P, sch.nslots], bf16, isOutput=False)
    iotar_in = nc.declare_dram_parameter("iotar", [P, slotmax * P], bf16, isOutput=False)
    dinvbc = nc.declare_dram_parameter("dinvbc", [P, nblk * P], f32, isOutput=False)
    dinvb = nc.declare_dram_parameter("dinvb", [P, nblk], f32, isOutput=False)
    dinvbg = nc.declare_dram_parameter("dinvbg", [P, sch.tblk_total], f32, isOutput=False)
    w1 = nc.declare_dram_parameter("W1", [in_dim, hid], bf16, isOutput=False)
    b1 = nc.declare_dram_parameter("b1", [hid, 1], f32, isOutput=False)
    w2 = nc.declare_dram_parameter("W2", [hid, out_dim], bf16, isOutput=False)
    b2bc = nc.declare_dram_parameter("b2bc", [P, out_dim], f32, isOutput=False)
    out_ext = nc.declare_dram_parameter("out", [shard, out_dim], f32, isOutput=True)

    hbank = [
        nc.dram_tensor(f"hbank{q}", [sch.bank_n[q], P], bf16) for q in range(4)
    ]
    h2loc_q = [
        nc.dram_tensor(f"h2loc{q}", [sch.qrows[q], P], bf16) for q in range(4)
    ]
    h2bank = [
        nc.dram_tensor(f"h2bank{q}", [sch.bank_n[q], P], bf16, addr_space="Shared")
        for q in range(4)
    ]

    kin = in_dim // P

    def quarter_of(b):
        for q, (b0, b1) in enumerate(qb):
            if b0 <= b < b1:
                return q
        raise AssertionError

    seg_gathers = {}
    for g in sch.gathers:
        seg_gathers.setdefault((g["sb"], g["bank"]), []).append(g)
    g_slots = {}
    for sl in sch.slots:
        g_slots.setdefault(sl["g"], []).append(sl)

    ag2_at_sb = {}
    for q in range(4):
        sq_end = (qb[q][1] - 1) // SB_N
        key = min(sq_end + AG2_LAG, n_sb - 1) if q < 3 else n_sb - 1
        ag2_at_sb.setdefault(key, []).append(q)

    with tile.TileContext(nc) as tc:
        with (
            tc.tile_pool(name="const", bufs=1) as cpool,
            tc.tile_pool(name="xload", bufs=2) as xpool,
            tc.tile_pool(name="hb", bufs=2) as hbpool,
            tc.tile_pool(name="idx", bufs=10) as ipool,
            tc.tile_pool(name="gath", bufs=GBUFS) as gpool,
            tc.tile_pool(name="sind", bufs=SBUFS) as spool,
            tc.tile_pool(name="dvp", bufs=12) as dvpool,
            tc.tile_pool(name="blk", bufs=3) as bpool,
            tc.tile_pool(name="psh", bufs=1, space="PSUM") as psh,
            tc.tile_pool(name="psagg", bufs=6, space="PSUM") as psagg,
            tc.tile_pool(name="psh2", bufs=1, space="PSUM") as psh2,
        ):
            import contextlib

            regstack = contextlib.ExitStack()
            nidx_vals = sorted({g["nidx"] for g in sch.gathers})
            nreg_map = {}
            for v in nidx_vals:
                r = regstack.enter_context(nc.gpsimd.register(f"nreg_{v}"))
                nc.gpsimd.reg_mov(r, v)
                nreg_map[v] = r

            # ---- constants into SBUF
            w1_t = [
                cpool.tile([P, hid], bf16, tag=f"w1_{k}", name=f"w1t{k}")
                for k in range(kin)
            ]
            for k in range(kin):
                nc.sync.dma_start(out=w1_t[k][:], in_=w1[k * P : (k + 1) * P, :])
            w2_sb = cpool.tile([hid, out_dim], bf16, tag="w2")
            nc.sync.dma_start(out=w2_sb[:], in_=w2[:])
            b1_sb = cpool.tile([hid, 1], f32, tag="b1")
            nc.sync.dma_start(out=b1_sb[:], in_=b1[:])
            b2_sb = cpool.tile([P, out_dim], f32, tag="b2")
            nc.sync.dma_start(out=b2_sb[:], in_=b2bc[:])
            dinvb_sb = cpool.tile([P, nblk], f32, tag="dinvb")
            nc.sync.dma_start(out=dinvb_sb[:], in_=dinvb[:])
            dinvbg_sb = cpool.tile([P, sch.tblk_total], f32, tag="dinvbg")
            nc.sync.dma_start(out=dinvbg_sb[:], in_=dinvbg[:])
            dstloc_sb = cpool.tile([P, sch.nslots], bf16, tag="dstloc")
            nc.sync.dma_start(out=dstloc_sb[:], in_=dstloc[:])
            iotar_sb = cpool.tile([P, slotmax * P], bf16, tag="iotar")
            nc.sync.dma_start(out=iotar_sb[:], in_=iotar_in[:])

            # ---- replicated h1: h~ = dinv * (x @ W1) for ALL nodes, written
            # bank-major directly into the bank tables (no collective).
            toff = 0  # global table row offset
            gblk = 0  # global table block index (dinvbg column)
            for q in range(4):
                bn = sch.bank_n[q]
                nb = sch.tblk_per_bank[q]
                for g0 in range(0, nb, XGRP):
                    g1 = min(g0 + XGRP, nb)
                    r0, r1 = g0 * P, min(g1 * P, bn)
                    xt = [
                        xpool.tile([P, XGRP * P], bf16, tag=f"xt{k}", name=f"xt{k}")
                        for k in range(kin)
                    ]
                    for k in range(kin):
                        nc.sync.dma_start(
                            out=xt[k][:, : r1 - r0],
                            in_=xT[k * P : (k + 1) * P, toff + r0 : toff + r1],
                        )
                    hsb = hbpool.tile([P, XGRP, hid], bf16, tag="hsb")
                    nfull = 0
                    for b in range(g0, g1):
                        m = min(P, bn - b * P)
                        hp = psh.tile([P, hid], f32, tag="hps")
                        for k in range(kin):
                            nc.tensor.matmul(
                                out=hp[:m, :],
                                lhsT=xt[k][:, b * P - r0 : b * P - r0 + m],
                                rhs=w1_t[k][:],
                                start=(k == 0),
                                stop=(k == kin - 1),
                            )
                        nc.scalar.activation(
                            out=hsb[:m, b - g0, :],
                            in_=hp[:m, :],
                            func=mybir.ActivationFunctionType.Copy,
                            scale=dinvbg_sb[:m, gblk + b : gblk + b + 1],
                        )
                        if m == P:
                            nfull += 1
                    if nfull:
                        nc.sync.dma_start(
                            out=hbank[q][r0 : r0 + nfull * P, :].rearrange(
                                "(g p) f -> p g f", p=P
                            ),
                            in_=hsb[:, :nfull, :],
                        )
                    if r1 - r0 > nfull * P:  # partial tail block
                        m = r1 - r0 - nfull * P
                        nc.sync.dma_start(
                            out=hbank[q][r0 + nfull * P : r1, :],
                            in_=hsb[:m, nfull, :],
                        )
                gblk += nb
                toff += bn

            # ---- layer pipelines
            def run_layer(layer):
                table = hbank if layer == 1 else h2bank
                w = P if layer == 1 else out_dim

                def prefetch_sb(s):
                    if s >= n_sb:
                        return {}
                    tiles = {}
                    for k in range(4):
                        for g in seg_gathers.get((s, k), []):
                            it = ipool.tile([P, gchmax * 8], mybir.dt.int16, tag="it")
                            nc.sync.dma_start(
                                out=it[:, : g["nidx"] // 16],
                                in_=idxs[:, g["c16"] : g["c16"] + g["nidx"] // 16],
                            )
                            tiles[g["gi"]] = it
                    return tiles

                it_tiles = prefetch_sb(0)
                for s in range(n_sb):
                    blocks = list(range(s * SB_N, min((s + 1) * SB_N, nblk)))
                    # prefetch next sb's idx tiles and this sb's dv tiles
                    # BEFORE this sb's epilogue stores hit the sync queue
                    next_tiles = prefetch_sb(s + 1)
                    dv_tiles = {}
                    if layer == 1:
                        for b in blocks:
                            dv = dvpool.tile([P, P], f32, tag="dv")
                            nc.sync.dma_start(
                                out=dv[:], in_=dinvbc[:, b * P : (b + 1) * P]
                            )
                            dv_tiles[b] = dv
                    agg_t = {
                        b: psagg.tile([P, w], f32, tag="agg", name=f"agg{layer}_{s}_{b}")
                        for b in blocks
                    }
                    for k in range(4):
                        for g in seg_gathers.get((s, k), []):
                            nidx, nch = g["nidx"], g["nch"]
                            it = it_tiles[g["gi"]]
                            gt = gpool.tile([P, gchmax, P], bf16, tag="gt")
                            nc.gpsimd.dma_gather(
                                out_ap=gt[:, :nch, :],
                                in_ap=table[k][0 : sch.bank_n[k], :],
                                idxs_ap=it[:, : nidx // 16],
                                num_idxs=nidx,
                                num_idxs_reg=nreg_map[nidx],
                                elem_size=P,
                                single_packet=False,
                                queue_num=k,
                            )
                            nsl = g["nslots"]
                            sbig = spool.tile([P, slotmax, P], bf16, tag="sind")
                            nc.vector.tensor_tensor(
                                out=sbig[:, :nsl, :],
                                in0=iotar_sb[:, : nsl * P].rearrange(
                                    "p (k f) -> p k f", k=nsl
                                ),
                                in1=dstloc_sb[
                                    :, g["slot0"] : g["slot0"] + nsl
                                ].to_broadcast([P, nsl, P]),
                                op=mybir.AluOpType.is_equal,
                            )
                            for sl in g_slots.get(g["gi"], []):
                                if layer == 1:
                                    nc.tensor.matmul(
                                        out=agg_t[sl["blk"]][:, :],
                                        lhsT=gt[:, sl["cl"], :],
                                        rhs=sbig[:, sl["sl_local"], :],
                                        start=sl["start"],
                                        stop=sl["stop"],
                                    )
                                else:
                                    nc.tensor.matmul(
                                        out=agg_t[sl["blk"]][:, :],
                                        lhsT=sbig[:, sl["sl_local"], :],
                                        rhs=gt[:, sl["cl"], :out_dim],
                                        start=sl["start"],
                                        stop=sl["stop"],
                                    )
                    # ---- block epilogues
                    for b in blocks:
                        m = min(P, shard - b * P)
                        if layer == 1:
                            t1 = bpool.tile([P, P], bf16, tag="t1")
                            nc.vector.tensor_tensor(
                                out=t1[:],
                                in0=agg_t[b][:, :],
                                in1=dv_tiles[b][:],
                                op=mybir.AluOpType.mult,
                            )
                            o1 = bpool.tile([P, P], bf16, tag="o1")
                            nc.scalar.activation(
                                out=o1[:],
                                in_=t1[:],
                                func=mybir.ActivationFunctionType.Relu,
                                bias=b1_sb[:, :1],
                            )
                            h2p = psh2.tile([P, out_dim], f32, tag="h2p")
                            nc.tensor.matmul(
                                out=h2p[:],
                                lhsT=o1[:],
                                rhs=w2_sb[:],
                                start=True,
                                stop=True,
                            )
                            h2s = bpool.tile([P, P], bf16, tag="h2s")
                            nc.scalar.activation(
                                out=h2s[:m, :out_dim],
                                in_=h2p[:m, :],
                                func=mybir.ActivationFunctionType.Copy,
                                scale=dinvb_sb[:m, b : b + 1],
                            )
                            q = quarter_of(b)
                            r0 = b * P - sch.qrow_start[q]
                            nc.sync.dma_start(
                                out=h2loc_q[q][r0 : r0 + m, 0:out_dim],
                                in_=h2s[:m, :out_dim],
                            )
                        else:
                            t2 = bpool.tile([P, out_dim], f32, tag="t2")
                            nc.scalar.activation(
                                out=t2[:m, :],
                                in_=agg_t[b][:m, :],
                                func=mybir.ActivationFunctionType.Copy,
                                scale=dinvb_sb[:m, b : b + 1],
                            )
                            ob = bpool.tile([P, out_dim], f32, tag="ob")
                            nc.vector.tensor_tensor(
                                out=ob[:m, :],
                                in0=t2[:m, :],
                                in1=b2_sb[:m, :],
                                op=mybir.AluOpType.add,
                            )
                            nc.sync.dma_start(
                                out=out_ext[b * P : b * P + m, :], in_=ob[:m, :]
                            )
                    if layer == 1:
                        for q in ag2_at_sb.get(s, []):
                            nc.gpsimd.collective_compute(
                                "AllGather",
                                mybir.AluOpType.bypass,
                                ins=[h2loc_q[q][:]],
                                outs=[h2bank[q][:]],
                                replica_groups=[list(range(NCORES))],
                            )
                    it_tiles = next_tiles

            run_layer(1)
            run_layer(2)
            regstack.close()

    nc.compile()
    return nc


# ---------------------------------------------------------------- kernel ---
def _make_in_maps(sch, x, W1, b1v, W2, b2v):
    hid = W1.shape[1]
    out_dim = W2.shape[1]
    shard, nblk = sch.shard, sch.nblk
    bf = ml_dtypes.bfloat16
    in_maps = []
    w1b = W1.astype(bf)
    w2b = W2.astype(bf)
    b1c = b1v.reshape(hid, 1).astype(np.float32).copy()
    b2c = np.broadcast_to(b2v.astype(np.float32), (P, out_dim)).copy()
    iotar = np.tile(np.arange(P, dtype=np.float32), (P, sch.slotmax)).astype(bf)
    # shared (identical across cores): permuted full xT and global dinv cols
    xs = np.ascontiguousarray(x[sch.table_nodes].astype(bf).T)
    dg = sch.dinv[sch.table_nodes]
    dgp = np.zeros(sch.tblk_total * P, np.float32)
    dgp[: dg.shape[0]] = dg
    dinvbg = np.ascontiguousarray(dgp.reshape(sch.tblk_total, P).T)
    for c in range(NCORES):
        dv = sch.dinv[c * shard : (c + 1) * shard]
        full = np.zeros(nblk * P, np.float32)
        full[:shard] = dv
        dvb = np.ascontiguousarray(full.reshape(nblk, P).T)
        dbc = np.broadcast_to(full, (P, nblk * P)).copy()
        in_maps.append(
            {
                "xT": xs,
                "idxs": sch.idx_stream[c],
                "dstloc": sch.dstloc_s[c],
                "dinvb": dvb,
                "dinvbg": dinvbg,
                "W1": w1b,
                "b1": b1c,
                "W2": w2b,
                "b2bc": b2c,
                "iotar": iotar,
                "dinvbc": dbc,
            }
        )
    return in_maps


def _get_compiled(n, e, edge_index, in_dim, hid, out_dim):
    key = ("nc", n, e)
    if key not in _CACHE:
        sch = _preprocess(n, edge_index)
        _CACHE[("sched", n, e)] = sch
        _CACHE[key] = _build(sch, in_dim, hid, out_dim)
    return _CACHE[("sched", n, e)], _CACHE[key]


def kernel(x, edge_index, W1, b1, W2, b2):
    _install_compat()
    from concourse.bass_utils import run_bass_kernel_spmd

    x = np.asarray(x)
    edge_index = np.asarray(edge_index)
    W1 = np.asarray(W1, np.float32)
    b1v = np.asarray(b1, np.float32)
    W2 = np.asarray(W2, np.float32)
    b2v = np.asarray(b2, np.float32)
    n, in_dim = x.shape
    hid = W1.shape[1]
    out_dim = W2.shape[1]

    sch, nc = _get_compiled(n, edge_index.shape[1], edge_index, in_dim, hid, out_dim)
    in_maps = _make_in_maps(sch, x, W1, b1v, W2, b2v)
    import os

    trace = bool(os.environ.get("GCN_TRACE"))
    res = run_bass_kernel_spmd(
        nc, in_maps, core_ids=list(range(NCORES)), trace=trace
    )
    global LAST_EXEC_NS
    LAST_EXEC_NS = res.exec_time_ns
    return np.concatenate([res.results[c]["out"] for c in range(NCORES)], axis=0)


LAST_EXEC_NS = None


# revision 16
# speedup vs baseline: 1.2993x; 1.2993x over previous
"""2-layer GCN (GCNConv x2) on 8 Trainium2 NeuronCores.

Strategy (dst-sharded, edge-partitioned by destination; Q7-descgen-optimized):
- Each core owns N/8 destination nodes and the edges pointing at them
  (plus the GCN self-loops, kept in the edge stream).
- Table layout: 4 banks, bank q = concat over cores of quarter-q of their
  shard (block-aligned quarters, int16-indexable). Layer 1's table
  h~ = dinv * (x @ W1) is computed REPLICATED on every core (x is a
  shared input; ~83us of idle PE + sequential DMA), written straight
  into the bank tensors in bank-major order so bank q is ready at
  ~(q+1)/4 of the h1 phase -- no AllGather and no collective-serialization
  in the critical path.
- Per (super-block of 6 dst blocks, bank): edges packed contiguously in
  block order into 128-slot chunks (straddling block boundaries); one
  dma_gather per segment on queue=bank (4 SWDGE queue contexts, balanced).
  Scatter-add via is_equal-indicator matmuls accumulating in PSUM.
- idx tiles and epilogue dv tiles are prefetched one super-block ahead so
  the sync-engine FIFO (epilogue stores) never blocks gather dispatch.
- Layer 1 accumulates transposed (aggT [feat, dst]) so bias+ReLU ride the
  activation engine and out1 feeds h2 = out1 @ W2 directly as lhsT;
  h2~ = dinv * h2 goes out bf16 through 4 bank-wise AllGathers placed
  as their quarters complete, so layer 2's gathers pipeline behind
  layer 1 and only the last bank pays a collective latency.
"""
import sys
import types

import numpy as np
import ml_dtypes

P = 128
NCORES = 8
GMAX = 32  # max chunks (128 idxs each) per dma_gather
SB_N = 5  # dst blocks per super-block (one PSUM bank each; 5+2+1 banks)
NQUEUES = 4
XGRP = 8
GBUFS = 10
SBUFS = 6
AG2_LAG = 2  # super-blocks between a quarter finishing and its AG2 trigger

_CACHE = {}


# ---------------------------------------------------------------- compat ---
def _install_compat():
    """Patches for this axon/walrus stack (drain waits, per-inst wait caps,
    NTFF shim). Idempotent."""
    if _CACHE.get("compat"):
        return
    import concourse.tile as tile
    import concourse.mybir as mybir

    _ev = [0]

    def _split_inst_waits(ordered):
        for _bb, insts in ordered.items():
            out = []
            for inst in insts:
                si = getattr(inst, "sync_info", None)
                if si is not None and si.on_wait is not None and len(si.on_wait) > 1:
                    waits = list(si.on_wait)
                    excess, keep = waits[:-1], waits[-1:]
                    si.on_wait.clear()
                    for sw in keep:
                        si.on_wait.append(sw)
                    for i in range(0, len(excess), 2):
                        _ev[0] += 1
                        ev = mybir.InstEventSemaphore(
                            name=f"evsplit-{_ev[0]}", ins=[], outs=[]
                        )
                        ev.engine = inst.engine
                        ev.sync_info = mybir.SyncInfo(
                            on_wait=excess[i : i + 2], on_update=[]
                        )
                        out.append(ev)
                out.append(inst)
            insts[:] = out

    orig_lower = tile.TileContext._lower_ordered_insts

    def patched_lower(self, ordered):
        _split_inst_waits(ordered)
        return orig_lower(self, ordered)

    def patched_drain(self, tick_clock, wait_clock):
        sems_alloc = list(self.sems.allocated().values())
        carrier = self.nc.sync.wait_ge(sems_alloc[0], 0)
        wait_clock.add_sem_waits(
            carrier.ins, tile.ScopedClock({None: tick_clock.global_clock})
        )
        waits = list(carrier.ins.sync_info.on_wait)
        carrier.ins.sync_info.on_wait.clear()
        for sw in waits[:2]:
            carrier.ins.sync_info.on_wait.append(sw)
        for i in range(2, len(waits), 2):
            c = self.nc.sync.wait_ge(sems_alloc[0], 0)
            c.ins.sync_info.on_wait.clear()
            for sw in waits[i : i + 2]:
                c.ins.sync_info.on_wait.append(sw)
        self.nc.sync.drain(fusable=False)
        self.nc.all_engine_barrier()
        popped = self.nc._tile_sem_poison_stack.pop()
        assert popped is self._sem_poison
        self.nc.clear_and_free_semaphores(sems_alloc)
        self.nc.all_engine_barrier()

    tile.TileContext._lower_ordered_insts = patched_lower
    tile.TileContext._drain_and_barrier = patched_drain

    # NTFF profile hook shim (missing antenv.axon_hooks in this image)
    _hook = {}
    mod = types.ModuleType("antenv.axon_hooks")
    mod.set_axon_ntff_profile_hook = lambda h: _hook.update(hook=h)
    mod.get_axon_ntff_profile_hook = lambda: _hook.get("hook")
    sys.modules["antenv.axon_hooks"] = mod
    try:
        import antenv

        antenv.axon_hooks = mod
        from trn_agent_boot.trn_boot import _ntff_profile_via_ctypes

        mod.set_axon_ntff_profile_hook(
            _ntff_profile_via_ctypes("/opt/axon/libaxon_pjrt.so")
        )
    except Exception:
        pass
    _CACHE["compat"] = True


# ---------------------------------------------------------- preprocessing ---
class Schedule:
    pass


def _quarter_bounds(nblk):
    base, rem = nblk // 4, nblk % 4
    sizes = [base + (1 if i < rem else 0) for i in range(4)]
    starts = np.cumsum([0] + sizes)
    return [(int(starts[i]), int(starts[i + 1])) for i in range(4)]


def _preprocess(n, edge_index):
    src0 = np.asarray(edge_index[0], np.int64)
    dst0 = np.asarray(edge_index[1], np.int64)
    shard = n // NCORES
    nblk = (shard + P - 1) // P
    n_sb = (nblk + SB_N - 1) // SB_N
    qb = _quarter_bounds(nblk)
    qrow_start = [b0 * P for b0, b1 in qb]
    qrows = [min(b1 * P, shard) - b0 * P for b0, b1 in qb]
    bank_n = [NCORES * r for r in qrows]
    assert all(b <= 32767 for b in bank_n)

    deg = np.bincount(dst0, minlength=n).astype(np.float64) + 1.0
    dinv = (1.0 / np.sqrt(deg)).astype(np.float32)

    # append GCN self-loops as regular edges
    loops = np.arange(n, dtype=np.int64)
    src = np.concatenate([src0, loops])
    dst = np.concatenate([dst0, loops])
    e = src.shape[0]

    core_d = dst // shard
    dl = dst - core_d * shard
    blk = dl // P
    dstloc = (dl % P).astype(np.int64)
    sb = blk // SB_N
    core_s = src // shard
    off = src - core_s * shard
    sblk = off // P
    qb_arr = np.zeros(nblk, np.int64)
    for q, (b0, b1) in enumerate(qb):
        qb_arr[b0:b1] = q
    bank = qb_arr[sblk]
    bidx = (
        core_s * np.array(qrows)[bank] + (off - np.array(qrow_start)[bank])
    ).astype(np.int64)

    order = np.lexsort((blk, bank, sb, core_d))
    s_core = core_d[order]
    s_sb = sb[order]
    s_bank = bank[order]
    s_blk = blk[order]
    s_bidx = bidx[order]
    s_dstloc = dstloc[order]

    key = ((s_core * n_sb + s_sb) * 4 + s_bank) * nblk + s_blk
    cnt = np.bincount(key, minlength=NCORES * n_sb * 4 * nblk).reshape(
        NCORES, n_sb, 4, nblk
    )

    gathers = []
    slots = []
    first_slot_of_block = {}
    last_slot_of_block = {}
    chunk_gid = 0
    slot_gid = 0
    budget_tab = np.zeros((n_sb, 4), np.int64)
    for s in range(n_sb):
        blocks = list(range(s * SB_N, min((s + 1) * SB_N, nblk)))
        for k in range(4):
            percore = cnt[:, s, k, :][:, blocks]
            cum = np.cumsum(percore, axis=1)
            budget = max(int(np.ceil(cum[:, -1].max() / P)), 1)
            budget_tab[s, k] = budget
            lo = np.min(cum - percore, axis=0)
            hi = np.max(cum, axis=0)
            g0 = 0
            while g0 < budget:
                gn = min(GMAX, budget - g0)
                gi = len(gathers)
                gslot0 = slot_gid
                for j in range(g0, g0 + gn):
                    c_lo, c_hi = j * P, (j + 1) * P
                    for bi, b in enumerate(blocks):
                        if hi[bi] > c_lo and lo[bi] < c_hi:
                            slots.append(
                                dict(
                                    g=gi,
                                    cl=j - g0,
                                    blk=b,
                                    sb=s,
                                    bank=k,
                                    sl_local=slot_gid - gslot0,
                                    chunk_gid=chunk_gid + j,
                                )
                            )
                            first_slot_of_block.setdefault((s, b), slot_gid)
                            last_slot_of_block[(s, b)] = slot_gid
                            slot_gid += 1
                gathers.append(
                    dict(
                        gi=gi,
                        sb=s,
                        bank=k,
                        chunk0=chunk_gid + g0,
                        nch=gn,
                        nidx=gn * P,
                        slot0=gslot0,
                        nslots=slot_gid - gslot0,
                        c16=(chunk_gid + g0) * P // 16,
                    )
                )
                g0 += gn
            chunk_gid += budget
    totc = chunk_gid
    nslots = slot_gid
    tot_slots = totc * P
    for i, sl in enumerate(slots):
        sl["start"] = first_slot_of_block[(sl["sb"], sl["blk"])] == i
        sl["stop"] = last_slot_of_block[(sl["sb"], sl["blk"])] == i
    slotmax = max(g["nslots"] for g in gathers)
    gchmax = max(g["nch"] for g in gathers)

    seg_key = (s_core * n_sb + s_sb) * 4 + s_bank
    seg_ptr = np.searchsorted(seg_key, np.arange(NCORES * n_sb * 4 + 1))
    seg_chunk0 = {}
    cg = 0
    for s in range(n_sb):
        for k in range(4):
            seg_chunk0[(s, k)] = cg
            cg += int(budget_tab[s, k])

    idx_flat = np.zeros((NCORES, tot_slots), np.int16)
    dstloc_s = np.full((NCORES, P, nslots), -1.0, np.float32)
    for c in range(NCORES):
        arr = idx_flat[c]
        for s in range(n_sb):
            for k in range(4):
                p0 = seg_ptr[(c * n_sb + s) * 4 + k]
                p1 = seg_ptr[(c * n_sb + s) * 4 + k + 1]
                base = seg_chunk0[(s, k)] * P
                arr[base : base + (p1 - p0)] = s_bidx[p0:p1].astype(np.int16)
                # pads stay 0 (gather row 0; indicator -1 masks them out).

    seg_id = (s_core * n_sb + s_sb) * 4 + s_bank
    pos_in_seg = np.arange(e) - seg_ptr[seg_id]
    seg_chunk0_arr = np.zeros(NCORES * n_sb * 4, np.int64)
    for s in range(n_sb):
        for k in range(4):
            for c in range(NCORES):
                seg_chunk0_arr[(c * n_sb + s) * 4 + k] = seg_chunk0[(s, k)]
    chunk_of_edge = seg_chunk0_arr[seg_id] + pos_in_seg // P
    part_of_edge = pos_in_seg % P
    slot_lut = np.full((totc, nblk), -1, np.int64)
    for i, sl in enumerate(slots):
        slot_lut[sl["chunk_gid"], sl["blk"]] = i
    slot_of_edge = slot_lut[chunk_of_edge, s_blk]
    assert (slot_of_edge >= 0).all()
    dstloc_s[s_core, part_of_edge, slot_of_edge] = s_dstloc

    # wrap idx stream: slot i -> [lane i%16, col i//16], replicate to 128 parts
    idx_stream = np.ascontiguousarray(
        idx_flat.reshape(NCORES, tot_slots // 16, 16).transpose(0, 2, 1)
    )
    idx_stream = np.tile(idx_stream, (1, 8, 1))

    # table node order (bank-major) for the replicated h1 phase
    nodes = []
    for q in range(4):
        idx = (
            np.arange(NCORES)[:, None] * shard
            + qrow_start[q]
            + np.arange(qrows[q])[None, :]
        ).ravel()
        nodes.append(idx)
    table_nodes = np.concatenate(nodes)
    assert table_nodes.shape[0] == n
    tblk_per_bank = [-(-b // P) for b in bank_n]
    tblk_total = sum(tblk_per_bank)

    sch = Schedule()
    sch.n, sch.e, sch.shard, sch.nblk, sch.n_sb = n, e, shard, nblk, n_sb
    sch.qb, sch.qrow_start, sch.qrows, sch.bank_n = qb, qrow_start, qrows, bank_n
    sch.dinv = dinv
    sch.gathers = gathers
    sch.slots = slots
    sch.totc, sch.nslots, sch.tot_slots = totc, nslots, tot_slots
    sch.slotmax = slotmax
    sch.gchmax = gchmax
    sch.idx_stream = idx_stream
    sch.idx_flat = idx_flat
    sch.dstloc_s = dstloc_s.astype(ml_dtypes.bfloat16)
    sch.table_nodes = table_nodes
    sch.tblk_per_bank = tblk_per_bank
    sch.tblk_total = tblk_total
    return sch


# ----------------------------------------------------------------- build ---
def _build(sch, in_dim, hid, out_dim):
    import concourse.mybir as mybir
    import concourse.tile as tile
    from concourse import bacc

    bf16 = mybir.dt.bfloat16
    f32 = mybir.dt.float32
    shard, nblk, n_sb = sch.shard, sch.nblk, sch.n_sb
    slotmax, gchmax = sch.slotmax, sch.gchmax
    qb = sch.qb
    n = sch.n

    nc = bacc.Bacc(num_swdge_queues=NQUEUES)

    xT = nc.declare_dram_parameter("xT", [in_dim, n], bf16, isOutput=False)
    idxs = nc.declare_dram_parameter(
        "idxs", [P, sch.tot_slots // 16], mybir.dt.int16, isOutput=False
    )
    dstloc = nc.declare_dram_parameter("dstloc", [P, sch.nslots], bf16, isOutput=False)
    iotar_in = nc.declare_dram_parameter("iotar", [P, slotmax * P], bf16, isOutput=False)
    dinvbc = nc.declare_dram_parameter("dinvbc", [P, nblk * P], f32, isOutput=False)
    dinvb = nc.declare_dram_parameter("dinvb", [P, nblk], f32, isOutput=False)
    ones_in = nc.declare_dram_parameter("ones", [P, 1], f32, isOutput=False)
    w1 = nc.declare_dram_parameter("W1", [in_dim, hid], bf16, isOutput=False)
    b1 = nc.declare_dram_parameter("b1", [hid, 1], f32, isOutput=False)
    w2 = nc.declare_dram_parameter("W2", [hid, out_dim], bf16, isOutput=False)
    b2bc = nc.declare_dram_parameter("b2bc", [P, out_dim], f32, isOutput=False)
    out_ext = nc.declare_dram_parameter("out", [shard, out_dim], f32, isOutput=True)

    hbank = [
        nc.dram_tensor(f"hbank{q}", [sch.bank_n[q], P], bf16) for q in range(4)
    ]
    h2loc_q = [
        nc.dram_tensor(f"h2loc{q}", [sch.qrows[q], P], bf16) for q in range(4)
    ]
    h2bank = [
        nc.dram_tensor(f"h2bank{q}", [sch.bank_n[q], P], bf16, addr_space="Shared")
        for q in range(4)
    ]

    kin = in_dim // P

    def quarter_of(b):
        for q, (b0, b1) in enumerate(qb):
            if b0 <= b < b1:
                return q
        raise AssertionError

    seg_gathers = {}
    for g in sch.gathers:
        seg_gathers.setdefault((g["sb"], g["bank"]), []).append(g)
    g_slots = {}
    for sl in sch.slots:
        g_slots.setdefault(sl["g"], []).append(sl)

    ag2_at_sb = {}
    for q in range(4):
        sq_end = (qb[q][1] - 1) // SB_N
        key = min(sq_end + AG2_LAG, n_sb - 1) if q < 3 else n_sb - 1
        ag2_at_sb.setdefault(key, []).append(q)

    with tile.TileContext(nc) as tc:
        with (
            tc.tile_pool(name="const", bufs=1) as cpool,
            tc.tile_pool(name="xload", bufs=2) as xpool,
            tc.tile_pool(name="hb", bufs=2) as hbpool,
            tc.tile_pool(name="idx", bufs=10) as ipool,
            tc.tile_pool(name="gath", bufs=GBUFS) as gpool,
            tc.tile_pool(name="sind", bufs=SBUFS) as spool,
            tc.tile_pool(name="dvp", bufs=12) as dvpool,
            tc.tile_pool(name="blk", bufs=3) as bpool,
            tc.tile_pool(name="psh", bufs=2, space="PSUM") as psh,
            tc.tile_pool(name="psagg", bufs=SB_N, space="PSUM") as psagg,
            tc.tile_pool(name="psh2", bufs=1, space="PSUM") as psh2,
        ):
            import contextlib

            regstack = contextlib.ExitStack()
            nidx_vals = sorted({g["nidx"] for g in sch.gathers})
            nreg_map = {}
            for v in nidx_vals:
                r = regstack.enter_context(nc.gpsimd.register(f"nreg_{v}"))
                nc.gpsimd.reg_mov(r, v)
                nreg_map[v] = r

            # ---- constants into SBUF
            w1_t = [
                cpool.tile([P, hid], bf16, tag=f"w1_{k}", name=f"w1t{k}")
                for k in range(kin)
            ]
            for k in range(kin):
                nc.sync.dma_start(out=w1_t[k][:], in_=w1[k * P : (k + 1) * P, :])
            w2_sb = cpool.tile([hid, out_dim], bf16, tag="w2")
            nc.sync.dma_start(out=w2_sb[:], in_=w2[:])
            b1_sb = cpool.tile([hid, 1], f32, tag="b1")
            nc.sync.dma_start(out=b1_sb[:], in_=b1[:])
            b2_sb = cpool.tile([P, out_dim], f32, tag="b2")
            nc.sync.dma_start(out=b2_sb[:], in_=b2bc[:])
            dinvb_sb = cpool.tile([P, nblk], f32, tag="dinvb")
            nc.sync.dma_start(out=dinvb_sb[:], in_=dinvb[:])
            ones_sb = cpool.tile([P, 1], f32, tag="ones")
            nc.sync.dma_start(out=ones_sb[:], in_=ones_in[:])
            dstloc_sb = cpool.tile([P, sch.nslots], bf16, tag="dstloc")
            nc.sync.dma_start(out=dstloc_sb[:], in_=dstloc[:])
            iotar_sb = cpool.tile([P, slotmax * P], bf16, tag="iotar")
            nc.sync.dma_start(out=iotar_sb[:], in_=iotar_in[:])

            # ---- replicated h1: h~ = (dinv*x) @ W1 for ALL nodes (x is
            # host-prescaled by dinv), written bank-major directly into the
            # bank tables (no collective). 4 blocks batched per PSUM bank;
            # PSUM->SBUF copies alternate ACT / DVE(tensor_tensor x 1.0).
            toff = 0  # global table row offset
            copy_alt = [0]
            for q in range(4):
                bn = sch.bank_n[q]
                nb = sch.tblk_per_bank[q]
                for g0 in range(0, nb, XGRP):
                    g1 = min(g0 + XGRP, nb)
                    r0, r1 = g0 * P, min(g1 * P, bn)
                    xt = [
                        xpool.tile([P, XGRP * P], bf16, tag=f"xt{k}", name=f"xt{k}")
                        for k in range(kin)
                    ]
                    for k in range(kin):
                        nc.sync.dma_start(
                            out=xt[k][:, : r1 - r0],
                            in_=xT[k * P : (k + 1) * P, toff + r0 : toff + r1],
                        )
                    hsb = hbpool.tile([P, XGRP, hid], bf16, tag="hsb")
                    nfull = 0
                    for s0 in range(g0, g1, 4):
                        s1 = min(s0 + 4, g1)
                        hp = psh.tile([P, 4 * hid], f32, tag="hps")
                        for b in range(s0, s1):
                            m = min(P, bn - b * P)
                            if m == P:
                                nfull += 1
                            sub = b - s0
                            for k in range(kin):
                                nc.tensor.matmul(
                                    out=hp[:m, sub * hid : (sub + 1) * hid],
                                    lhsT=xt[k][:, b * P - r0 : b * P - r0 + m],
                                    rhs=w1_t[k][:],
                                    start=(k == 0),
                                    stop=(k == kin - 1),
                                )
                        nsub = s1 - s0
                        if copy_alt[0] % 2 == 0:
                            nc.scalar.activation(
                                out=hsb[:, s0 - g0 : s0 - g0 + nsub, :],
                                in_=hp[:, : nsub * hid].rearrange(
                                    "p (g f) -> p g f", g=nsub
                                ),
                                func=mybir.ActivationFunctionType.Copy,
                            )
                        else:
                            nc.vector.tensor_tensor(
                                out=hsb[:, s0 - g0 : s0 - g0 + nsub, :],
                                in0=hp[:, : nsub * hid].rearrange(
                                    "p (g f) -> p g f", g=nsub
                                ),
                                in1=ones_sb[:, 0:1].to_broadcast([P, nsub, hid]),
                                op=mybir.AluOpType.mult,
                            )
                        copy_alt[0] += 1
                    if nfull:
                        nc.sync.dma_start(
                            out=hbank[q][r0 : r0 + nfull * P, :].rearrange(
                                "(g p) f -> p g f", p=P
                            ),
                            in_=hsb[:, :nfull, :],
                        )
                    if r1 - r0 > nfull * P:  # partial tail block
                        m = r1 - r0 - nfull * P
                        nc.sync.dma_start(
                            out=hbank[q][r0 + nfull * P : r1, :],
                            in_=hsb[:m, nfull, :],
                        )
                toff += bn

            # ---- layer pipelines
            def run_layer(layer):
                table = hbank if layer == 1 else h2bank
                w = P if layer == 1 else out_dim

                def prefetch_sb(s):
                    if s >= n_sb:
                        return {}
                    tiles = {}
                    for k in range(4):
                        for g in seg_gathers.get((s, k), []):
                            it = ipool.tile([P, gchmax * 8], mybir.dt.int16, tag="it")
                            nc.sync.dma_start(
                                out=it[:, : g["nidx"] // 16],
                                in_=idxs[:, g["c16"] : g["c16"] + g["nidx"] // 16],
                            )
                            tiles[g["gi"]] = it
                    return tiles

                it_tiles = prefetch_sb(0)
                for s in range(n_sb):
                    blocks = list(range(s * SB_N, min((s + 1) * SB_N, nblk)))
                    # prefetch next sb's idx tiles and this sb's dv tiles
                    # BEFORE this sb's epilogue stores hit the sync queue
                    next_tiles = prefetch_sb(s + 1)
                    dv_tiles = {}
                    if layer == 1:
                        for b in blocks:
                            dv = dvpool.tile([P, P], f32, tag="dv")
                            nc.sync.dma_start(
                                out=dv[:], in_=dinvbc[:, b * P : (b + 1) * P]
                            )
                            dv_tiles[b] = dv
                    agg_t = {
                        b: psagg.tile([P, w], f32, tag="agg", name=f"agg{layer}_{s}_{b}")
                        for b in blocks
                    }
                    for k in range(4):
                        for g in seg_gathers.get((s, k), []):
                            nidx, nch = g["nidx"], g["nch"]
                            it = it_tiles[g["gi"]]
                            gt = gpool.tile([P, gchmax, P], bf16, tag="gt")
                            nc.gpsimd.dma_gather(
                                out_ap=gt[:, :nch, :],
                                in_ap=table[k][0 : sch.bank_n[k], :],
                                idxs_ap=it[:, : nidx // 16],
                                num_idxs=nidx,
                                num_idxs_reg=nreg_map[nidx],
                                elem_size=P,
                                single_packet=False,
                                queue_num=k,
                            )
                            nsl = g["nslots"]
                            sbig = spool.tile([P, slotmax, P], bf16, tag="sind")
                            nc.vector.tensor_tensor(
                                out=sbig[:, :nsl, :],
                                in0=iotar_sb[:, : nsl * P].rearrange(
                                    "p (k f) -> p k f", k=nsl
                                ),
                                in1=dstloc_sb[
                                    :, g["slot0"] : g["slot0"] + nsl
                                ].to_broadcast([P, nsl, P]),
                                op=mybir.AluOpType.is_equal,
                            )
                            for sl in g_slots.get(g["gi"], []):
                                if layer == 1:
                                    nc.tensor.matmul(
                                        out=agg_t[sl["blk"]][:, :],
                                        lhsT=gt[:, sl["cl"], :],
                                        rhs=sbig[:, sl["sl_local"], :],
                                        start=sl["start"],
                                        stop=sl["stop"],
                                    )
                                else:
                                    nc.tensor.matmul(
                                        out=agg_t[sl["blk"]][:, :],
                                        lhsT=sbig[:, sl["sl_local"], :],
                                        rhs=gt[:, sl["cl"], :out_dim],
                                        start=sl["start"],
                                        stop=sl["stop"],
                                    )
                    # ---- block epilogues
                    for b in blocks:
                        m = min(P, shard - b * P)
                        if layer == 1:
                            t1 = bpool.tile([P, P], bf16, tag="t1")
                            nc.vector.tensor_tensor(
                                out=t1[:],
                                in0=agg_t[b][:, :],
                                in1=dv_tiles[b][:],
                                op=mybir.AluOpType.mult,
                            )
                            o1 = bpool.tile([P, P], bf16, tag="o1")
                            nc.scalar.activation(
                                out=o1[:],
                                in_=t1[:],
                                func=mybir.ActivationFunctionType.Relu,
                                bias=b1_sb[:, :1],
                            )
                            h2p = psh2.tile([P, out_dim], f32, tag="h2p")
                            nc.tensor.matmul(
                                out=h2p[:],
                                lhsT=o1[:],
                                rhs=w2_sb[:],
                                start=True,
                                stop=True,
                            )
                            h2s = bpool.tile([P, P], bf16, tag="h2s")
                            nc.scalar.activation(
                                out=h2s[:m, :out_dim],
                                in_=h2p[:m, :],
                                func=mybir.ActivationFunctionType.Copy,
                                scale=dinvb_sb[:m, b : b + 1],
                            )
                            q = quarter_of(b)
                            r0 = b * P - sch.qrow_start[q]
                            nc.sync.dma_start(
                                out=h2loc_q[q][r0 : r0 + m, 0:out_dim],
                                in_=h2s[:m, :out_dim],
                            )
                        else:
                            t2 = bpool.tile([P, out_dim], f32, tag="t2")
                            nc.scalar.activation(
                                out=t2[:m, :],
                                in_=agg_t[b][:m, :],
                                func=mybir.ActivationFunctionType.Copy,
                                scale=dinvb_sb[:m, b : b + 1],
                            )
                            ob = bpool.tile([P, out_dim], f32, tag="ob")
                            nc.vector.tensor_tensor(
                                out=ob[:m, :],
                                in0=t2[:m, :],
                                in1=b2_sb[:m, :],
                                op=mybir.AluOpType.add,
                            )
                            nc.sync.dma_start(
                                out=out_ext[b * P : b * P + m, :], in_=ob[:m, :]
                            )
                    if layer == 1:
                        for q in ag2_at_sb.get(s, []):
                            nc.gpsimd.collective_compute(
                                "AllGather",
                                mybir.AluOpType.bypass,
                                ins=[h2loc_q[q][:]],
                                outs=[h2bank[q][:]],
                                replica_groups=[list(range(NCORES))],
                            )
                    it_tiles = next_tiles

            run_layer(1)
            run_layer(2)
            regstack.close()

    nc.compile()
    return nc


# ---------------------------------------------------------------- kernel ---
def _make_in_maps(sch, x, W1, b1v, W2, b2v):
    hid = W1.shape[1]
    out_dim = W2.shape[1]
    shard, nblk = sch.shard, sch.nblk
    bf = ml_dtypes.bfloat16
    in_maps = []
    w1b = W1.astype(bf)
    w2b = W2.astype(bf)
    b1c = b1v.reshape(hid, 1).astype(np.float32).copy()
    b2c = np.broadcast_to(b2v.astype(np.float32), (P, out_dim)).copy()
    iotar = np.tile(np.arange(P, dtype=np.float32), (P, sch.slotmax)).astype(bf)
    ones = np.ones((P, 1), np.float32)
    # shared (identical across cores): permuted, dinv-prescaled full xT
    xs = np.ascontiguousarray(
        (x[sch.table_nodes] * sch.dinv[sch.table_nodes][:, None]).astype(bf).T
    )
    for c in range(NCORES):
        dv = sch.dinv[c * shard : (c + 1) * shard]
        full = np.zeros(nblk * P, np.float32)
        full[:shard] = dv
        dvb = np.ascontiguousarray(full.reshape(nblk, P).T)
        dbc = np.broadcast_to(full, (P, nblk * P)).copy()
        in_maps.append(
            {
                "xT": xs,
                "idxs": sch.idx_stream[c],
                "dstloc": sch.dstloc_s[c],
                "dinvb": dvb,
                "ones": ones,
                "W1": w1b,
                "b1": b1c,
                "W2": w2b,
                "b2bc": b2c,
                "iotar": iotar,
                "dinvbc": dbc,
            }
        )
    return in_maps


def _get_compiled(n, e, edge_index, in_dim, hid, out_dim):
    key = ("nc", n, e)
    if key not in _CACHE:
        sch = _preprocess(n, edge_index)
        _CACHE[("sched", n, e)] = sch
        _CACHE[key] = _build(sch, in_dim, hid, out_dim)
    return _CACHE[("sched", n, e)], _CACHE[key]


def kernel(x, edge_index, W1, b1, W2, b2):
    _install_compat()
    from concourse.bass_utils import run_bass_kernel_spmd

    x = np.asarray(x)
    edge_index = np.asarray(edge_index)
    W1 = np.asarray(W1, np.float32)
    b1v = np.asarray(b1, np.float32)
    W2 = np.asarray(W2, np.float32)
    b2v = np.asarray(b2, np.float32)
    n, in_dim = x.shape
    hid = W1.shape[1]
    out_dim = W2.shape[1]

    sch, nc = _get_compiled(n, edge_index.shape[1], edge_index, in_dim, hid, out_dim)
    in_maps = _make_in_maps(sch, x, W1, b1v, W2, b2v)
    import os

    trace = bool(os.environ.get("GCN_TRACE"))
    res = run_bass_kernel_spmd(
        nc, in_maps, core_ids=list(range(NCORES)), trace=trace
    )
    global LAST_EXEC_NS
    LAST_EXEC_NS = res.exec_time_ns
    return np.concatenate([res.results[c]["out"] for c in range(NCORES)], axis=0)


LAST_EXEC_NS = None


# revision 27
# speedup vs baseline: 1.4533x; 1.1186x over previous
"""2-layer GCN (GCNConv x2) on 8 Trainium2 NeuronCores.

Strategy (dst-sharded, edge-partitioned by destination; Q7-descgen-optimized):
- Each core owns N/8 destination nodes and the edges pointing at them
  (plus the GCN self-loops, kept in the edge stream).
- Layer-1 table: h~ = (dinv*x) @ W1 computed per-shard (x host-prescaled by
  dinv), ONE AllGather into hfull; gather banks = 4 int16-addressable row
  slices of hfull (rank-pair layout). A single collective minimizes the
  ~60-95us per-op CC-stream overhead of this stack.
- Layer-2 table: h2~ = dinv * h2 written per-quarter (block-aligned
  quarters), 4 bank-wise AllGathers fired as quarters complete so they
  overlap layer-1's gather phase; bank q = concat over cores of quarter q.
- Since the two layers use different bank layouts, each layer has its own
  edge schedule (idx stream / dstloc / slot table); the dst-side blocking
  is shared.
- Per (super-block of 5 dst blocks, bank): edges packed contiguously in
  block order into 128-slot chunks (straddling block boundaries); one
  dma_gather per segment on queue=bank (4 SWDGE queue contexts, balanced).
  Scatter-add via is_equal-indicator matmuls accumulating in PSUM.
- idx tiles and epilogue dv tiles prefetched one super-block ahead so the
  sync-engine FIFO (epilogue stores) never delays gather dispatch; layer 2's
  first bank-0..2 gathers are pre-issued BEFORE the last AG2 trigger
  (collective triggers block the gpsimd SEQ until completion).
- Layer 1 accumulates transposed (aggT [feat, dst]) so bias+ReLU ride the
  activation engine and out1 feeds h2 = out1 @ W2 directly as lhsT.
"""
import sys
import types

import numpy as np
import ml_dtypes

P = 128
NCORES = 8
GMAX = 32  # max chunks (128 idxs each) per dma_gather
SB_N = 5  # dst blocks per super-block (one PSUM bank each; 5+2+1 banks)
NQUEUES = 4
XGRP = 8
GBUFS = 10
SBUFS = 7
AG2_LAG = 2  # super-blocks between a quarter finishing and its AG2 trigger

_CACHE = {}


# ---------------------------------------------------------------- compat ---
def _install_compat():
    """Patches for this axon/walrus stack (drain waits, per-inst wait caps,
    NTFF shim). Idempotent."""
    if _CACHE.get("compat"):
        return
    import concourse.tile as tile
    import concourse.mybir as mybir

    _ev = [0]

    def _split_inst_waits(ordered):
        for _bb, insts in ordered.items():
            out = []
            for inst in insts:
                si = getattr(inst, "sync_info", None)
                if si is not None and si.on_wait is not None and len(si.on_wait) > 1:
                    waits = list(si.on_wait)
                    excess, keep = waits[:-1], waits[-1:]
                    si.on_wait.clear()
                    for sw in keep:
                        si.on_wait.append(sw)
                    for i in range(0, len(excess), 2):
                        _ev[0] += 1
                        ev = mybir.InstEventSemaphore(
                            name=f"evsplit-{_ev[0]}", ins=[], outs=[]
                        )
                        ev.engine = inst.engine
                        ev.sync_info = mybir.SyncInfo(
                            on_wait=excess[i : i + 2], on_update=[]
                        )
                        out.append(ev)
                out.append(inst)
            insts[:] = out

    orig_lower = tile.TileContext._lower_ordered_insts

    def patched_lower(self, ordered):
        _split_inst_waits(ordered)
        return orig_lower(self, ordered)

    def patched_drain(self, tick_clock, wait_clock):
        sems_alloc = list(self.sems.allocated().values())
        carrier = self.nc.sync.wait_ge(sems_alloc[0], 0)
        wait_clock.add_sem_waits(
            carrier.ins, tile.ScopedClock({None: tick_clock.global_clock})
        )
        waits = list(carrier.ins.sync_info.on_wait)
        carrier.ins.sync_info.on_wait.clear()
        for sw in waits[:2]:
            carrier.ins.sync_info.on_wait.append(sw)
        for i in range(2, len(waits), 2):
            c = self.nc.sync.wait_ge(sems_alloc[0], 0)
            c.ins.sync_info.on_wait.clear()
            for sw in waits[i : i + 2]:
                c.ins.sync_info.on_wait.append(sw)
        self.nc.sync.drain(fusable=False)
        self.nc.all_engine_barrier()
        popped = self.nc._tile_sem_poison_stack.pop()
        assert popped is self._sem_poison
        self.nc.clear_and_free_semaphores(sems_alloc)
        self.nc.all_engine_barrier()

    tile.TileContext._lower_ordered_insts = patched_lower
    tile.TileContext._drain_and_barrier = patched_drain

    # NTFF profile hook shim (missing antenv.axon_hooks in this image)
    _hook = {}
    mod = types.ModuleType("antenv.axon_hooks")
    mod.set_axon_ntff_profile_hook = lambda h: _hook.update(hook=h)
    mod.get_axon_ntff_profile_hook = lambda: _hook.get("hook")
    sys.modules["antenv.axon_hooks"] = mod
    try:
        import antenv

        antenv.axon_hooks = mod
        from trn_agent_boot.trn_boot import _ntff_profile_via_ctypes

        mod.set_axon_ntff_profile_hook(
            _ntff_profile_via_ctypes("/opt/axon/libaxon_pjrt.so")
        )
    except Exception:
        pass
    _CACHE["compat"] = True


# ---------------------------------------------------------- preprocessing ---
class Schedule:
    pass


class LayerSched:
    pass


def _quarter_bounds(nblk):
    base, rem = nblk // 4, nblk % 4
    sizes = [base + (1 if i < rem else 0) for i in range(4)]
    starts = np.cumsum([0] + sizes)
    return [(int(starts[i]), int(starts[i + 1])) for i in range(4)]


def _make_layer_sched(
    n, nblk, n_sb, s_core, s_sb, s_bank, s_blk, s_bidx, s_dstloc
):
    """Build the per-(super-block, bank) straddle-packed schedule for one
    bank mapping. Inputs are edge arrays sorted by (core, sb, bank, blk)."""
    e = s_core.shape[0]
    key = ((s_core * n_sb + s_sb) * 4 + s_bank) * nblk + s_blk
    cnt = np.bincount(key, minlength=NCORES * n_sb * 4 * nblk).reshape(
        NCORES, n_sb, 4, nblk
    )

    gathers = []
    slots = []
    first_slot_of_block = {}
    last_slot_of_block = {}
    chunk_gid = 0
    slot_gid = 0
    budget_tab = np.zeros((n_sb, 4), np.int64)
    for s in range(n_sb):
        blocks = list(range(s * SB_N, min((s + 1) * SB_N, nblk)))
        for k in range(4):
            percore = cnt[:, s, k, :][:, blocks]
            cum = np.cumsum(percore, axis=1)
            budget = max(int(np.ceil(cum[:, -1].max() / P)), 1)
            budget_tab[s, k] = budget
            lo = np.min(cum - percore, axis=0)
            hi = np.max(cum, axis=0)
            g0 = 0
            while g0 < budget:
                gn = min(GMAX, budget - g0)
                gi = len(gathers)
                gslot0 = slot_gid
                for j in range(g0, g0 + gn):
                    c_lo, c_hi = j * P, (j + 1) * P
                    for bi, b in enumerate(blocks):
                        if hi[bi] > c_lo and lo[bi] < c_hi:
                            slots.append(
                                dict(
                                    g=gi,
                                    cl=j - g0,
                                    blk=b,
                                    sb=s,
                                    bank=k,
                                    sl_local=slot_gid - gslot0,
                                    chunk_gid=chunk_gid + j,
                                )
                            )
                            first_slot_of_block.setdefault((s, b), slot_gid)
                            last_slot_of_block[(s, b)] = slot_gid
                            slot_gid += 1
                gathers.append(
                    dict(
                        gi=gi,
                        sb=s,
                        bank=k,
                        chunk0=chunk_gid + g0,
                        nch=gn,
                        nidx=gn * P,
                        slot0=gslot0,
                        nslots=slot_gid - gslot0,
                        c16=(chunk_gid + g0) * P // 16,
                    )
                )
                g0 += gn
            chunk_gid += budget
    totc = chunk_gid
    nslots = slot_gid
    tot_slots = totc * P
    # the self-loop identity matmul opens each block's PSUM group (start);
    # the last slot closes it (stop)
    for i, sl in enumerate(slots):
        sl["start"] = False
        sl["stop"] = last_slot_of_block[(sl["sb"], sl["blk"])] == i
    has_slots = set(first_slot_of_block.keys())

    seg_key = (s_core * n_sb + s_sb) * 4 + s_bank
    seg_ptr = np.searchsorted(seg_key, np.arange(NCORES * n_sb * 4 + 1))
    seg_chunk0 = {}
    cg = 0
    for s in range(n_sb):
        for k in range(4):
            seg_chunk0[(s, k)] = cg
            cg += int(budget_tab[s, k])

    idx_flat = np.zeros((NCORES, tot_slots), np.int16)
    for c in range(NCORES):
        arr = idx_flat[c]
        for s in range(n_sb):
            for k in range(4):
                p0 = seg_ptr[(c * n_sb + s) * 4 + k]
                p1 = seg_ptr[(c * n_sb + s) * 4 + k + 1]
                base = seg_chunk0[(s, k)] * P
                arr[base : base + (p1 - p0)] = s_bidx[p0:p1].astype(np.int16)

    dstloc_s = np.full((NCORES, P, nslots), -1.0, np.float32)
    seg_id = (s_core * n_sb + s_sb) * 4 + s_bank
    pos_in_seg = np.arange(e) - seg_ptr[seg_id]
    seg_chunk0_arr = np.zeros(NCORES * n_sb * 4, np.int64)
    for s in range(n_sb):
        for k in range(4):
            for c in range(NCORES):
                seg_chunk0_arr[(c * n_sb + s) * 4 + k] = seg_chunk0[(s, k)]
    chunk_of_edge = seg_chunk0_arr[seg_id] + pos_in_seg // P
    part_of_edge = pos_in_seg % P
    slot_lut = np.full((totc, nblk), -1, np.int64)
    for i, sl in enumerate(slots):
        slot_lut[sl["chunk_gid"], sl["blk"]] = i
    slot_of_edge = slot_lut[chunk_of_edge, s_blk]
    assert (slot_of_edge >= 0).all()
    dstloc_s[s_core, part_of_edge, slot_of_edge] = s_dstloc

    idx_stream = np.ascontiguousarray(
        idx_flat.reshape(NCORES, tot_slots // 16, 16).transpose(0, 2, 1)
    )
    idx_stream = np.tile(idx_stream, (1, 8, 1))

    ls = LayerSched()
    ls.has_slots = has_slots
    ls.gathers = gathers
    ls.slots = slots
    ls.totc, ls.nslots, ls.tot_slots = totc, nslots, tot_slots
    ls.slotmax = max(g["nslots"] for g in gathers)
    ls.gchmax = max(g["nch"] for g in gathers)
    ls.idx_stream = idx_stream
    ls.idx_flat = idx_flat
    ls.dstloc_s = dstloc_s.astype(ml_dtypes.bfloat16)
    return ls


def _preprocess(n, edge_index):
    src0 = np.asarray(edge_index[0], np.int64)
    dst0 = np.asarray(edge_index[1], np.int64)
    shard = n // NCORES
    nblk = (shard + P - 1) // P
    n_sb = (nblk + SB_N - 1) // SB_N
    qb = _quarter_bounds(nblk)
    qrow_start = [b0 * P for b0, b1 in qb]
    qrows = [min(b1 * P, shard) - b0 * P for b0, b1 in qb]
    bank2_n = [NCORES * r for r in qrows]
    assert all(b <= 32767 for b in bank2_n)
    bank1_rows = (n + 3) // 4  # rank-pair banks for layer 1 (slices of hfull)
    assert bank1_rows <= 32767

    deg = np.bincount(dst0, minlength=n).astype(np.float64) + 1.0
    dinv = (1.0 / np.sqrt(deg)).astype(np.float32)

    # self-loops are NOT in the edge stream: both layers fold them in with
    # one identity matmul per block from the per-core hloc/h2loc tensors.
    src = src0
    dst = dst0

    core_d = dst // shard
    dl = dst - core_d * shard
    blk = dl // P
    dstloc = (dl % P).astype(np.int64)
    sb = blk // SB_N

    # layer-1 bank mapping: contiguous row slices of hfull (rank-major)
    bank_a = src // bank1_rows
    bidx_a = src - bank_a * bank1_rows

    # layer-2 bank mapping: quarter-stacked
    core_s = src // shard
    off = src - core_s * shard
    sblk = off // P
    qb_arr = np.zeros(nblk, np.int64)
    for q, (b0, b1) in enumerate(qb):
        qb_arr[b0:b1] = q
    bank_b = qb_arr[sblk]
    bidx_b = core_s * np.array(qrows)[bank_b] + (off - np.array(qrow_start)[bank_b])

    scheds = []
    for bank, bidx in ((bank_a, bidx_a), (bank_b, bidx_b)):
        order = np.lexsort((blk, bank, sb, core_d))
        scheds.append(
            _make_layer_sched(
                n,
                nblk,
                n_sb,
                core_d[order],
                sb[order],
                bank[order],
                blk[order],
                bidx[order],
                dstloc[order],
            )
        )

    sch = Schedule()
    sch.n, sch.shard, sch.nblk, sch.n_sb = n, shard, nblk, n_sb
    sch.e = src.shape[0]
    sch.qb, sch.qrow_start, sch.qrows = qb, qrow_start, qrows
    sch.bank1_rows, sch.bank2_n = bank1_rows, bank2_n
    sch.dinv = dinv
    sch.L1, sch.L2 = scheds
    sch.slotmax = max(sch.L1.slotmax, sch.L2.slotmax)
    return sch


# ----------------------------------------------------------------- build ---
def _build(sch, in_dim, hid, out_dim):
    import concourse.mybir as mybir
    import concourse.tile as tile
    from concourse import bacc

    bf16 = mybir.dt.bfloat16
    f32 = mybir.dt.float32
    shard, nblk, n_sb = sch.shard, sch.nblk, sch.n_sb
    slotmax = sch.slotmax
    gchmax = max(sch.L1.gchmax, sch.L2.gchmax)
    qb = sch.qb
    n = sch.n

    nc = bacc.Bacc(num_swdge_queues=NQUEUES)

    xT = nc.declare_dram_parameter("xT", [in_dim, shard], bf16, isOutput=False)
    idxs1 = nc.declare_dram_parameter(
        "idxs1", [P, sch.L1.tot_slots // 16], mybir.dt.int16, isOutput=False
    )
    idxs2 = nc.declare_dram_parameter(
        "idxs2", [P, sch.L2.tot_slots // 16], mybir.dt.int16, isOutput=False
    )
    dstloc1 = nc.declare_dram_parameter(
        "dstloc1", [P, sch.L1.nslots], bf16, isOutput=False
    )
    dstloc2 = nc.declare_dram_parameter(
        "dstloc2", [P, sch.L2.nslots], bf16, isOutput=False
    )
    iotar_in = nc.declare_dram_parameter("iotar", [P, slotmax * P], bf16, isOutput=False)
    dinvbc = nc.declare_dram_parameter("dinvbc", [P, nblk * P], f32, isOutput=False)
    dinvb = nc.declare_dram_parameter("dinvb", [P, nblk], f32, isOutput=False)
    w1 = nc.declare_dram_parameter("W1", [in_dim, hid], bf16, isOutput=False)
    b1 = nc.declare_dram_parameter("b1", [hid, 1], f32, isOutput=False)
    w2 = nc.declare_dram_parameter("W2", [hid, out_dim], bf16, isOutput=False)
    b2bc = nc.declare_dram_parameter("b2bc", [P, out_dim], f32, isOutput=False)
    ident_in = nc.declare_dram_parameter("ident", [P, P], bf16, isOutput=False)
    out_ext = nc.declare_dram_parameter("out", [shard, out_dim], f32, isOutput=True)

    hloc = nc.dram_tensor("hloc", [shard, P], bf16)
    hfull = nc.dram_tensor("hfull", [n, P], bf16, addr_space="Shared")
    h2loc_q = [
        nc.dram_tensor(f"h2loc{q}", [sch.qrows[q], P], bf16) for q in range(4)
    ]
    h2bank = [
        nc.dram_tensor(f"h2bank{q}", [sch.bank2_n[q], P], bf16, addr_space="Shared")
        for q in range(4)
    ]

    kin = in_dim // P

    def quarter_of(b):
        for q, (b0, b1) in enumerate(qb):
            if b0 <= b < b1:
                return q
        raise AssertionError

    def layer_maps(ls):
        seg_gathers = {}
        for g in ls.gathers:
            seg_gathers.setdefault((g["sb"], g["bank"]), []).append(g)
        g_slots = {}
        for sl in ls.slots:
            g_slots.setdefault(sl["g"], []).append(sl)
        return seg_gathers, g_slots

    seg1, gsl1 = layer_maps(sch.L1)
    seg2, gsl2 = layer_maps(sch.L2)

    ag2_at_sb = {}
    for q in range(4):
        sq_end = (qb[q][1] - 1) // SB_N
        key = min(sq_end + AG2_LAG, n_sb - 1) if q < 3 else n_sb - 1
        ag2_at_sb.setdefault(key, []).append(q)

    # layer-2 gathers pre-issued before the final AG2 trigger (the trigger
    # blocks the gpsimd SEQ until the collective completes)
    l2_pre = [(0, 0), (1, 0), (0, 1), (1, 1), (0, 2), (1, 2)]

    with tile.TileContext(nc) as tc:
        with (
            tc.tile_pool(name="const", bufs=1) as cpool,
            tc.tile_pool(name="xload", bufs=2) as xpool,
            tc.tile_pool(name="hb", bufs=3) as hbpool,
            tc.tile_pool(name="idx", bufs=12) as ipool,
            tc.tile_pool(name="gath", bufs=GBUFS) as gpool,
            tc.tile_pool(name="sind", bufs=SBUFS) as spool,
            tc.tile_pool(name="dvp", bufs=12) as dvpool,
            tc.tile_pool(name="rl", bufs=12) as rpool,
            tc.tile_pool(name="blk", bufs=3) as bpool,
            tc.tile_pool(name="psh", bufs=2, space="PSUM") as psh,
            tc.tile_pool(name="psagg", bufs=SB_N, space="PSUM") as psagg,
            tc.tile_pool(name="psh2", bufs=1, space="PSUM") as psh2,
        ):
            import contextlib

            regstack = contextlib.ExitStack()
            nidx_vals = sorted(
                {g["nidx"] for g in sch.L1.gathers}
                | {g["nidx"] for g in sch.L2.gathers}
            )
            nreg_map = {}
            for v in nidx_vals:
                r = regstack.enter_context(nc.gpsimd.register(f"nreg_{v}"))
                nc.gpsimd.reg_mov(r, v)
                nreg_map[v] = r

            # ---- constants into SBUF
            w1_t = [
                cpool.tile([P, hid], bf16, tag=f"w1_{k}", name=f"w1t{k}")
                for k in range(kin)
            ]
            for k in range(kin):
                nc.sync.dma_start(out=w1_t[k][:], in_=w1[k * P : (k + 1) * P, :])
            w2_sb = cpool.tile([hid, out_dim], bf16, tag="w2")
            nc.sync.dma_start(out=w2_sb[:], in_=w2[:])
            b1_sb = cpool.tile([hid, 1], f32, tag="b1")
            nc.sync.dma_start(out=b1_sb[:], in_=b1[:])
            b2_sb = cpool.tile([P, out_dim], f32, tag="b2")
            nc.sync.dma_start(out=b2_sb[:], in_=b2bc[:])
            dinvb_sb = cpool.tile([P, nblk], f32, tag="dinvb")
            nc.sync.dma_start(out=dinvb_sb[:], in_=dinvb[:])
            dstloc1_sb = cpool.tile([P, sch.L1.nslots], bf16, tag="dstloc1")
            nc.sync.dma_start(out=dstloc1_sb[:], in_=dstloc1[:])
            dstloc2_sb = cpool.tile([P, sch.L2.nslots], bf16, tag="dstloc2")
            nc.sync.dma_start(out=dstloc2_sb[:], in_=dstloc2[:])
            iotar_sb = cpool.tile([P, slotmax * P], bf16, tag="iotar")
            nc.sync.dma_start(out=iotar_sb[:], in_=iotar_in[:])
            ident_sb = cpool.tile([P, P], bf16, tag="ident")
            nc.sync.dma_start(out=ident_sb[:], in_=ident_in[:])

            # ---- h~ = (dinv*x) @ W1, shard-local (x pre-scaled by dinv)
            for g0 in range(0, nblk, XGRP):
                g1 = min(g0 + XGRP, nblk)
                c0, c1 = g0 * P, min(g1 * P, shard)
                xt = [
                    xpool.tile([P, XGRP * P], bf16, tag=f"xt{k}", name=f"xt{k}")
                    for k in range(kin)
                ]
                for k in range(kin):
                    nc.sync.dma_start(
                        out=xt[k][:, : c1 - c0], in_=xT[k * P : (k + 1) * P, c0:c1]
                    )
                for b in range(g0, g1):
                    m = min(P, shard - b * P)
                    hp = psh.tile([P, hid], f32, tag="hps")
                    for k in range(kin):
                        nc.tensor.matmul(
                            out=hp[:m, :],
                            lhsT=xt[k][:, b * P - c0 : b * P - c0 + m],
                            rhs=w1_t[k][:],
                            start=(k == 0),
                            stop=(k == kin - 1),
                        )
                    hsb = hbpool.tile([P, hid], bf16, tag="hsb")
                    nc.scalar.activation(
                        out=hsb[:m, :],
                        in_=hp[:m, :],
                        func=mybir.ActivationFunctionType.Copy,
                    )
                    nc.sync.dma_start(out=hloc[b * P : b * P + m, :], in_=hsb[:m, :])

            nc.gpsimd.collective_compute(
                "AllGather",
                mybir.AluOpType.bypass,
                ins=[hloc[:]],
                outs=[hfull[:]],
                replica_groups=[list(range(NCORES))],
            )

            def bank_table(layer, k):
                if layer == 1:
                    r0 = k * sch.bank1_rows
                    r1 = min(r0 + sch.bank1_rows, n)
                    return hfull[r0:r1, :]
                return h2bank[k][0 : sch.bank2_n[k], :]

            def issue_gather(layer, g, it):
                nidx, nch = g["nidx"], g["nch"]
                gt = gpool.tile([P, gchmax, P], bf16, tag="gt")
                nc.gpsimd.dma_gather(
                    out_ap=gt[:, :nch, :],
                    in_ap=bank_table(layer, g["bank"]),
                    idxs_ap=it[:, : nidx // 16],
                    num_idxs=nidx,
                    num_idxs_reg=nreg_map[nidx],
                    elem_size=P,
                    single_packet=False,
                    queue_num=g["bank"],
                )
                return gt

            def load_it(layer, g):
                idxs = idxs1 if layer == 1 else idxs2
                it = ipool.tile([P, gchmax * 8], mybir.dt.int16, tag="it")
                nc.sync.dma_start(
                    out=it[:, : g["nidx"] // 16],
                    in_=idxs[:, g["c16"] : g["c16"] + g["nidx"] // 16],
                )
                return it

            pre_issued = {}  # (layer, gi) -> gt tile

            def run_layer(layer):
                seg_gathers = seg1 if layer == 1 else seg2
                g_slots = gsl1 if layer == 1 else gsl2
                ls = sch.L1 if layer == 1 else sch.L2
                dstloc_sb = dstloc1_sb if layer == 1 else dstloc2_sb
                w = P if layer == 1 else out_dim

                def prefetch_sb(s):
                    tiles = {}
                    if s >= n_sb:
                        return tiles
                    for k in range(4):
                        for g in seg_gathers.get((s, k), []):
                            if (layer, g["gi"]) in pre_issued:
                                continue
                            tiles[g["gi"]] = load_it(layer, g)
                    return tiles

                def prefetch_rl(s):
                    tiles = {}
                    if s >= n_sb:
                        return tiles
                    for b in range(s * SB_N, min((s + 1) * SB_N, nblk)):
                        m = min(P, shard - b * P)
                        rl = rpool.tile([P, P], bf16, tag="rl")
                        if layer == 1:
                            nc.sync.dma_start(
                                out=rl[:m, :], in_=hloc[b * P : b * P + m, :]
                            )
                        else:
                            q = quarter_of(b)
                            r0 = b * P - sch.qrow_start[q]
                            nc.sync.dma_start(
                                out=rl[:m, :], in_=h2loc_q[q][r0 : r0 + m, :]
                            )
                        tiles[b] = rl
                    return tiles

                it_tiles = prefetch_sb(0)
                rl_tiles = prefetch_rl(0)
                for s in range(n_sb):
                    blocks = list(range(s * SB_N, min((s + 1) * SB_N, nblk)))
                    next_tiles = prefetch_sb(s + 1)
                    next_rl = prefetch_rl(s + 1)
                    dv_tiles = {}
                    if layer == 1:
                        for b in blocks:
                            dv = dvpool.tile([P, P], f32, tag="dv")
                            nc.sync.dma_start(
                                out=dv[:], in_=dinvbc[:, b * P : (b + 1) * P]
                            )
                            dv_tiles[b] = dv
                    agg_t = {
                        b: psagg.tile([P, w], f32, tag="agg", name=f"agg{layer}_{s}_{b}")
                        for b in blocks
                    }
                    # self-loop contribution opens each block's PSUM group
                    for b in blocks:
                        m = min(P, shard - b * P)
                        rl = rl_tiles[b]
                        solo = (s, b) not in ls.has_slots
                        if layer == 1:
                            nc.tensor.matmul(
                                out=agg_t[b][:, :],
                                lhsT=rl[:m, :],
                                rhs=ident_sb[:m, :],
                                start=True,
                                stop=solo,
                            )
                        else:
                            nc.tensor.matmul(
                                out=agg_t[b][:, :],
                                lhsT=ident_sb[:m, :],
                                rhs=rl[:m, :out_dim],
                                start=True,
                                stop=solo,
                            )
                    for k in range(4):
                        for g in seg_gathers.get((s, k), []):
                            if (layer, g["gi"]) in pre_issued:
                                gt = pre_issued.pop((layer, g["gi"]))
                            else:
                                gt = issue_gather(layer, g, it_tiles[g["gi"]])
                            nsl = g["nslots"]
                            sbig = spool.tile([P, slotmax, P], bf16, tag="sind")
                            nc.vector.tensor_tensor(
                                out=sbig[:, :nsl, :],
                                in0=iotar_sb[:, : nsl * P].rearrange(
                                    "p (k f) -> p k f", k=nsl
                                ),
                                in1=dstloc_sb[
                                    :, g["slot0"] : g["slot0"] + nsl
                                ].to_broadcast([P, nsl, P]),
                                op=mybir.AluOpType.is_equal,
                            )
                            for sl in g_slots.get(g["gi"], []):
                                if layer == 1:
                                    nc.tensor.matmul(
                                        out=agg_t[sl["blk"]][:, :],
                                        lhsT=gt[:, sl["cl"], :],
                                        rhs=sbig[:, sl["sl_local"], :],
                                        start=sl["start"],
                                        stop=sl["stop"],
                                    )
                                else:
                                    nc.tensor.matmul(
                                        out=agg_t[sl["blk"]][:, :],
                                        lhsT=sbig[:, sl["sl_local"], :],
                                        rhs=gt[:, sl["cl"], :out_dim],
                                        start=sl["start"],
                                        stop=sl["stop"],
                                    )
                    # ---- block epilogues
                    for b in blocks:
                        m = min(P, shard - b * P)
                        if layer == 1:
                            t1 = bpool.tile([P, P], bf16, tag="t1")
                            nc.vector.tensor_tensor(
                                out=t1[:],
                                in0=agg_t[b][:, :],
                                in1=dv_tiles[b][:],
                                op=mybir.AluOpType.mult,
                            )
                            o1 = bpool.tile([P, P], bf16, tag="o1")
                            nc.scalar.activation(
                                out=o1[:],
                                in_=t1[:],
                                func=mybir.ActivationFunctionType.Relu,
                                bias=b1_sb[:, :1],
                            )
                            h2p = psh2.tile([P, out_dim], f32, tag="h2p")
                            nc.tensor.matmul(
                                out=h2p[:],
                                lhsT=o1[:],
                                rhs=w2_sb[:],
                                start=True,
                                stop=True,
                            )
                            h2s = bpool.tile([P, P], bf16, tag="h2s")
                            nc.scalar.activation(
                                out=h2s[:m, :out_dim],
                                in_=h2p[:m, :],
                                func=mybir.ActivationFunctionType.Copy,
                                scale=dinvb_sb[:m, b : b + 1],
                            )
                            q = quarter_of(b)
                            r0 = b * P - sch.qrow_start[q]
                            nc.sync.dma_start(
                                out=h2loc_q[q][r0 : r0 + m, 0:out_dim],
                                in_=h2s[:m, :out_dim],
                            )
                        else:
                            t2 = bpool.tile([P, out_dim], f32, tag="t2")
                            nc.scalar.activation(
                                out=t2[:m, :],
                                in_=agg_t[b][:m, :],
                                func=mybir.ActivationFunctionType.Copy,
                                scale=dinvb_sb[:m, b : b + 1],
                            )
                            ob = bpool.tile([P, out_dim], f32, tag="ob")
                            nc.vector.tensor_tensor(
                                out=ob[:m, :],
                                in0=t2[:m, :],
                                in1=b2_sb[:m, :],
                                op=mybir.AluOpType.add,
                            )
                            nc.sync.dma_start(
                                out=out_ext[b * P : b * P + m, :], in_=ob[:m, :]
                            )
                    rl_tiles = next_rl
                    if layer == 1:
                        trig = ag2_at_sb.get(s, [])
                        if 3 in trig:
                            # pre-issue layer-2 bank-0..2 gathers so the Q7
                            # queues stay busy while the final AG2 runs
                            for (s2, k2) in l2_pre:
                                for g in seg2.get((s2, k2), []):
                                    it = load_it(2, g)
                                    pre_issued[(2, g["gi"])] = issue_gather(2, g, it)
                        for q in trig:
                            nc.gpsimd.collective_compute(
                                "AllGather",
                                mybir.AluOpType.bypass,
                                ins=[h2loc_q[q][:]],
                                outs=[h2bank[q][:]],
                                replica_groups=[list(range(NCORES))],
                            )
                    it_tiles = next_tiles

            run_layer(1)
            run_layer(2)
            regstack.close()

    nc.compile()
    return nc


# ---------------------------------------------------------------- kernel ---
def _make_in_maps(sch, x, W1, b1v, W2, b2v):
    hid = W1.shape[1]
    out_dim = W2.shape[1]
    shard, nblk = sch.shard, sch.nblk
    bf = ml_dtypes.bfloat16
    in_maps = []
    w1b = W1.astype(bf)
    w2b = W2.astype(bf)
    b1c = b1v.reshape(hid, 1).astype(np.float32).copy()
    b2c = np.broadcast_to(b2v.astype(np.float32), (P, out_dim)).copy()
    iotar = np.tile(np.arange(P, dtype=np.float32), (P, sch.slotmax)).astype(bf)
    ident = np.eye(P, dtype=np.float32).astype(bf)
    xs_all = (x * sch.dinv[:, None]).astype(bf)
    for c in range(NCORES):
        xs = np.ascontiguousarray(xs_all[c * shard : (c + 1) * shard].T)
        dv = sch.dinv[c * shard : (c + 1) * shard]
        full = np.zeros(nblk * P, np.float32)
        full[:shard] = dv
        dvb = np.ascontiguousarray(full.reshape(nblk, P).T)
        dbc = np.broadcast_to(full, (P, nblk * P)).copy()
        in_maps.append(
            {
                "xT": xs,
                "idxs1": sch.L1.idx_stream[c],
                "idxs2": sch.L2.idx_stream[c],
                "dstloc1": sch.L1.dstloc_s[c],
                "dstloc2": sch.L2.dstloc_s[c],
                "dinvb": dvb,
                "W1": w1b,
                "b1": b1c,
                "W2": w2b,
                "b2bc": b2c,
                "iotar": iotar,
                "ident": ident,
                "dinvbc": dbc,
            }
        )
    return in_maps


def _get_compiled(n, e, edge_index, in_dim, hid, out_dim):
    key = ("nc", n, e)
    if key not in _CACHE:
        sch = _preprocess(n, edge_index)
        _CACHE[("sched", n, e)] = sch
        _CACHE[key] = _build(sch, in_dim, hid, out_dim)
    return _CACHE[("sched", n, e)], _CACHE[key]


def kernel(x, edge_index, W1, b1, W2, b2):
    _install_compat()
    from concourse.bass_utils import run_bass_kernel_spmd

    x = np.asarray(x)
    edge_index = np.asarray(edge_index)
    W1 = np.asarray(W1, np.float32)
    b1v = np.asarray(b1, np.float32)
    W2 = np.asarray(W2, np.float32)
    b2v = np.asarray(b2, np.float32)
    n, in_dim = x.shape
    hid = W1.shape[1]
    out_dim = W2.shape[1]

    sch, nc = _get_compiled(n, edge_index.shape[1], edge_index, in_dim, hid, out_dim)
    in_maps = _make_in_maps(sch, x, W1, b1v, W2, b2v)
    import os

    trace = bool(os.environ.get("GCN_TRACE"))
    res = run_bass_kernel_spmd(
        nc, in_maps, core_ids=list(range(NCORES)), trace=trace
    )
    global LAST_EXEC_NS
    LAST_EXEC_NS = res.exec_time_ns
    return np.concatenate([res.results[c]["out"] for c in range(NCORES)], axis=0)


LAST_EXEC_NS = None


# revision 33
# speedup vs baseline: 1.6079x; 1.1064x over previous
"""2-layer GCN (GCNConv x2) on 8 Trainium2 NeuronCores.

Strategy (dst-sharded, edge-partitioned by destination; Q7-descgen-optimized):
- Each core owns N/8 destination nodes and the edges pointing at them
  (plus the GCN self-loops, kept in the edge stream).
- Layer-1 table: h~ = (dinv*x) @ W1 computed per-shard (x host-prescaled by
  dinv), ONE AllGather into hfull; gather banks = 4 int16-addressable row
  slices of hfull (rank-pair layout). A single collective minimizes the
  ~60-95us per-op CC-stream overhead of this stack.
- Layer-2 table: h2~ = dinv * h2 written per-quarter (block-aligned
  quarters), 4 bank-wise AllGathers fired as quarters complete so they
  overlap layer-1's gather phase; bank q = concat over cores of quarter q.
- Since the two layers use different bank layouts, each layer has its own
  edge schedule (idx stream / dstloc / slot table); the dst-side blocking
  is shared.
- Per (super-block of 5 dst blocks, bank): edges packed contiguously in
  block order into 128-slot chunks (straddling block boundaries); one
  dma_gather per segment on queue=bank (4 SWDGE queue contexts, balanced).
  Scatter-add via is_equal-indicator matmuls accumulating in PSUM.
- idx tiles and epilogue dv tiles prefetched one super-block ahead so the
  sync-engine FIFO (epilogue stores) never delays gather dispatch; layer 2's
  first bank-0..2 gathers are pre-issued BEFORE the last AG2 trigger
  (collective triggers block the gpsimd SEQ until completion).
- Layer 1 accumulates transposed (aggT [feat, dst]) so bias+ReLU ride the
  activation engine and out1 feeds h2 = out1 @ W2 directly as lhsT.
"""
import sys
import types

import numpy as np
import ml_dtypes

P = 128
NCORES = 8
GMAX = 32  # max chunks (128 idxs each) per dma_gather
SB_N = 5  # dst blocks per super-block (one PSUM bank each; 5+2+1 banks)
NQUEUES = 4
XGRP = 8
GBUFS = 10
SBUFS = 9
AG2_LAG = 2  # super-blocks between a quarter finishing and its AG2 trigger

_CACHE = {}


# ---------------------------------------------------------------- compat ---
def _install_compat():
    """Patches for this axon/walrus stack (drain waits, per-inst wait caps,
    NTFF shim). Idempotent."""
    if _CACHE.get("compat"):
        return
    import concourse.tile as tile
    import concourse.mybir as mybir

    _ev = [0]

    def _split_inst_waits(ordered):
        for _bb, insts in ordered.items():
            out = []
            for inst in insts:
                si = getattr(inst, "sync_info", None)
                if si is not None and si.on_wait is not None and len(si.on_wait) > 1:
                    waits = list(si.on_wait)
                    excess, keep = waits[:-1], waits[-1:]
                    si.on_wait.clear()
                    for sw in keep:
                        si.on_wait.append(sw)
                    for i in range(0, len(excess), 2):
                        _ev[0] += 1
                        ev = mybir.InstEventSemaphore(
                            name=f"evsplit-{_ev[0]}", ins=[], outs=[]
                        )
                        ev.engine = inst.engine
                        ev.sync_info = mybir.SyncInfo(
                            on_wait=excess[i : i + 2], on_update=[]
                        )
                        out.append(ev)
                out.append(inst)
            insts[:] = out

    orig_lower = tile.TileContext._lower_ordered_insts

    def patched_lower(self, ordered):
        _split_inst_waits(ordered)
        return orig_lower(self, ordered)

    def patched_drain(self, tick_clock, wait_clock):
        sems_alloc = list(self.sems.allocated().values())
        carrier = self.nc.sync.wait_ge(sems_alloc[0], 0)
        wait_clock.add_sem_waits(
            carrier.ins, tile.ScopedClock({None: tick_clock.global_clock})
        )
        waits = list(carrier.ins.sync_info.on_wait)
        carrier.ins.sync_info.on_wait.clear()
        for sw in waits[:2]:
            carrier.ins.sync_info.on_wait.append(sw)
        for i in range(2, len(waits), 2):
            c = self.nc.sync.wait_ge(sems_alloc[0], 0)
            c.ins.sync_info.on_wait.clear()
            for sw in waits[i : i + 2]:
                c.ins.sync_info.on_wait.append(sw)
        self.nc.sync.drain(fusable=False)
        self.nc.all_engine_barrier()
        popped = self.nc._tile_sem_poison_stack.pop()
        assert popped is self._sem_poison
        self.nc.clear_and_free_semaphores(sems_alloc)
        self.nc.all_engine_barrier()

    tile.TileContext._lower_ordered_insts = patched_lower
    tile.TileContext._drain_and_barrier = patched_drain

    # NTFF profile hook shim (missing antenv.axon_hooks in this image)
    _hook = {}
    mod = types.ModuleType("antenv.axon_hooks")
    mod.set_axon_ntff_profile_hook = lambda h: _hook.update(hook=h)
    mod.get_axon_ntff_profile_hook = lambda: _hook.get("hook")
    sys.modules["antenv.axon_hooks"] = mod
    try:
        import antenv

        antenv.axon_hooks = mod
        from trn_agent_boot.trn_boot import _ntff_profile_via_ctypes

        mod.set_axon_ntff_profile_hook(
            _ntff_profile_via_ctypes("/opt/axon/libaxon_pjrt.so")
        )
    except Exception:
        pass
    _CACHE["compat"] = True


# ---------------------------------------------------------- preprocessing ---
class Schedule:
    pass


class LayerSched:
    pass


def _quarter_bounds(nblk):
    base, rem = nblk // 4, nblk % 4
    sizes = [base + (1 if i < rem else 0) for i in range(4)]
    starts = np.cumsum([0] + sizes)
    return [(int(starts[i]), int(starts[i + 1])) for i in range(4)]


def _make_layer_sched(
    n, nblk, n_sb, s_core, s_sb, s_bank, s_blk, s_bidx, s_dstloc
):
    """Build the per-(super-block, bank) straddle-packed schedule for one
    bank mapping. Inputs are edge arrays sorted by (core, sb, bank, blk)."""
    e = s_core.shape[0]
    key = ((s_core * n_sb + s_sb) * 4 + s_bank) * nblk + s_blk
    cnt = np.bincount(key, minlength=NCORES * n_sb * 4 * nblk).reshape(
        NCORES, n_sb, 4, nblk
    )

    gathers = []
    slots = []
    first_slot_of_block = {}
    last_slot_of_block = {}
    chunk_gid = 0
    slot_gid = 0
    budget_tab = np.zeros((n_sb, 4), np.int64)
    for s in range(n_sb):
        blocks = list(range(s * SB_N, min((s + 1) * SB_N, nblk)))
        for k in range(4):
            percore = cnt[:, s, k, :][:, blocks]
            cum = np.cumsum(percore, axis=1)
            budget = max(int(np.ceil(cum[:, -1].max() / P)), 1)
            budget_tab[s, k] = budget
            lo = np.min(cum - percore, axis=0)
            hi = np.max(cum, axis=0)
            g0 = 0
            while g0 < budget:
                gn = min(GMAX, budget - g0)
                gi = len(gathers)
                gslot0 = slot_gid
                for j in range(g0, g0 + gn):
                    c_lo, c_hi = j * P, (j + 1) * P
                    for bi, b in enumerate(blocks):
                        if hi[bi] > c_lo and lo[bi] < c_hi:
                            slots.append(
                                dict(
                                    g=gi,
                                    cl=j - g0,
                                    blk=b,
                                    sb=s,
                                    bank=k,
                                    sl_local=slot_gid - gslot0,
                                    chunk_gid=chunk_gid + j,
                                )
                            )
                            first_slot_of_block.setdefault((s, b), slot_gid)
                            last_slot_of_block[(s, b)] = slot_gid
                            slot_gid += 1
                gathers.append(
                    dict(
                        gi=gi,
                        sb=s,
                        bank=k,
                        chunk0=chunk_gid + g0,
                        nch=gn,
                        nidx=gn * P,
                        slot0=gslot0,
                        nslots=slot_gid - gslot0,
                        c16=(chunk_gid + g0) * P // 16,
                    )
                )
                g0 += gn
            chunk_gid += budget
    totc = chunk_gid
    nslots = slot_gid
    tot_slots = totc * P
    # the self-loop identity matmul opens each block's PSUM group (start);
    # the last slot closes it (stop)
    for i, sl in enumerate(slots):
        sl["start"] = False
        sl["stop"] = last_slot_of_block[(sl["sb"], sl["blk"])] == i
    has_slots = set(first_slot_of_block.keys())

    seg_key = (s_core * n_sb + s_sb) * 4 + s_bank
    seg_ptr = np.searchsorted(seg_key, np.arange(NCORES * n_sb * 4 + 1))
    seg_chunk0 = {}
    cg = 0
    for s in range(n_sb):
        for k in range(4):
            seg_chunk0[(s, k)] = cg
            cg += int(budget_tab[s, k])

    idx_flat = np.zeros((NCORES, tot_slots), np.int16)
    for c in range(NCORES):
        arr = idx_flat[c]
        for s in range(n_sb):
            for k in range(4):
                p0 = seg_ptr[(c * n_sb + s) * 4 + k]
                p1 = seg_ptr[(c * n_sb + s) * 4 + k + 1]
                base = seg_chunk0[(s, k)] * P
                arr[base : base + (p1 - p0)] = s_bidx[p0:p1].astype(np.int16)

    dstloc_s = np.full((NCORES, P, nslots), -1.0, np.float32)
    seg_id = (s_core * n_sb + s_sb) * 4 + s_bank
    pos_in_seg = np.arange(e) - seg_ptr[seg_id]
    seg_chunk0_arr = np.zeros(NCORES * n_sb * 4, np.int64)
    for s in range(n_sb):
        for k in range(4):
            for c in range(NCORES):
                seg_chunk0_arr[(c * n_sb + s) * 4 + k] = seg_chunk0[(s, k)]
    chunk_of_edge = seg_chunk0_arr[seg_id] + pos_in_seg // P
    part_of_edge = pos_in_seg % P
    slot_lut = np.full((totc, nblk), -1, np.int64)
    for i, sl in enumerate(slots):
        slot_lut[sl["chunk_gid"], sl["blk"]] = i
    slot_of_edge = slot_lut[chunk_of_edge, s_blk]
    assert (slot_of_edge >= 0).all()
    dstloc_s[s_core, part_of_edge, slot_of_edge] = s_dstloc

    idx_stream = np.ascontiguousarray(
        idx_flat.reshape(NCORES, tot_slots // 16, 16).transpose(0, 2, 1)
    )
    idx_stream = np.tile(idx_stream, (1, 8, 1))

    ls = LayerSched()
    ls.has_slots = has_slots
    ls.gathers = gathers
    ls.slots = slots
    ls.totc, ls.nslots, ls.tot_slots = totc, nslots, tot_slots
    ls.slotmax = max(g["nslots"] for g in gathers)
    ls.gchmax = max(g["nch"] for g in gathers)
    ls.idx_stream = idx_stream
    ls.idx_flat = idx_flat
    ls.dstloc_s = dstloc_s.astype(ml_dtypes.bfloat16)
    return ls


def _preprocess(n, edge_index):
    src0 = np.asarray(edge_index[0], np.int64)
    dst0 = np.asarray(edge_index[1], np.int64)
    shard = n // NCORES
    nblk = (shard + P - 1) // P
    n_sb = (nblk + SB_N - 1) // SB_N
    qb = _quarter_bounds(nblk)
    qrow_start = [b0 * P for b0, b1 in qb]
    qrows = [min(b1 * P, shard) - b0 * P for b0, b1 in qb]
    bank2_n = [NCORES * r for r in qrows]
    assert all(b <= 32767 for b in bank2_n)
    bank1_rows = (n + 3) // 4  # rank-pair banks for layer 1 (slices of hfull)
    assert bank1_rows <= 32767

    deg = np.bincount(dst0, minlength=n).astype(np.float64) + 1.0
    dinv = (1.0 / np.sqrt(deg)).astype(np.float32)

    # self-loops are NOT in the edge stream: both layers fold them in with
    # one identity matmul per block from the per-core hloc/h2loc tensors.
    src = src0
    dst = dst0

    core_d = dst // shard
    dl = dst - core_d * shard
    blk = dl // P
    dstloc = (dl % P).astype(np.int64)
    sb = blk // SB_N

    # layer-1 bank mapping: contiguous row slices of hfull (rank-major)
    bank_a = src // bank1_rows
    bidx_a = src - bank_a * bank1_rows

    # layer-2 bank mapping: quarter-stacked
    core_s = src // shard
    off = src - core_s * shard
    sblk = off // P
    qb_arr = np.zeros(nblk, np.int64)
    for q, (b0, b1) in enumerate(qb):
        qb_arr[b0:b1] = q
    bank_b = qb_arr[sblk]
    bidx_b = core_s * np.array(qrows)[bank_b] + (off - np.array(qrow_start)[bank_b])

    scheds = []
    for bank, bidx in ((bank_a, bidx_a), (bank_b, bidx_b)):
        order = np.lexsort((blk, bank, sb, core_d))
        scheds.append(
            _make_layer_sched(
                n,
                nblk,
                n_sb,
                core_d[order],
                sb[order],
                bank[order],
                blk[order],
                bidx[order],
                dstloc[order],
            )
        )

    sch = Schedule()
    sch.n, sch.shard, sch.nblk, sch.n_sb = n, shard, nblk, n_sb
    sch.e = src.shape[0]
    sch.qb, sch.qrow_start, sch.qrows = qb, qrow_start, qrows
    sch.bank1_rows, sch.bank2_n = bank1_rows, bank2_n
    sch.dinv = dinv
    sch.L1, sch.L2 = scheds
    sch.slotmax = max(sch.L1.slotmax, sch.L2.slotmax)
    return sch


# ----------------------------------------------------------------- build ---
def _build(sch, in_dim, hid, out_dim):
    import concourse.mybir as mybir
    import concourse.tile as tile
    from concourse import bacc

    bf16 = mybir.dt.bfloat16
    f32 = mybir.dt.float32
    shard, nblk, n_sb = sch.shard, sch.nblk, sch.n_sb
    slotmax = sch.slotmax
    gchmax = max(sch.L1.gchmax, sch.L2.gchmax)
    qb = sch.qb
    n = sch.n

    nc = bacc.Bacc(num_swdge_queues=NQUEUES)

    xT = nc.declare_dram_parameter("xT", [in_dim, shard], bf16, isOutput=False)
    idxs1 = nc.declare_dram_parameter(
        "idxs1", [P, sch.L1.tot_slots // 16], mybir.dt.int16, isOutput=False
    )
    idxs2 = nc.declare_dram_parameter(
        "idxs2", [P, sch.L2.tot_slots // 16], mybir.dt.int16, isOutput=False
    )
    dstloc1 = nc.declare_dram_parameter(
        "dstloc1", [P, sch.L1.nslots], bf16, isOutput=False
    )
    dstloc2 = nc.declare_dram_parameter(
        "dstloc2", [P, sch.L2.nslots], bf16, isOutput=False
    )
    iotar_in = nc.declare_dram_parameter("iotar", [P, slotmax * P], bf16, isOutput=False)
    dinvbc = nc.declare_dram_parameter("dinvbc", [P, nblk * P], f32, isOutput=False)
    dinvb = nc.declare_dram_parameter("dinvb", [P, nblk], f32, isOutput=False)
    w1 = nc.declare_dram_parameter("W1", [in_dim, hid], bf16, isOutput=False)
    b1 = nc.declare_dram_parameter("b1", [hid, 1], f32, isOutput=False)
    w2 = nc.declare_dram_parameter("W2", [hid, out_dim], bf16, isOutput=False)
    b2bc = nc.declare_dram_parameter("b2bc", [P, out_dim], f32, isOutput=False)
    ident_in = nc.declare_dram_parameter("ident", [P, P], bf16, isOutput=False)
    out_ext = nc.declare_dram_parameter("out", [shard, out_dim], f32, isOutput=True)

    hloc = nc.dram_tensor("hloc", [shard, P], bf16)
    hfull = nc.dram_tensor("hfull", [n, P], bf16, addr_space="Shared")
    h2loc_q = [
        nc.dram_tensor(f"h2loc{q}", [sch.qrows[q], P], bf16) for q in range(4)
    ]
    h2bank = [
        nc.dram_tensor(f"h2bank{q}", [sch.bank2_n[q], P], bf16, addr_space="Shared")
        for q in range(4)
    ]

    kin = in_dim // P

    def quarter_of(b):
        for q, (b0, b1) in enumerate(qb):
            if b0 <= b < b1:
                return q
        raise AssertionError

    def layer_maps(ls):
        seg_gathers = {}
        for g in ls.gathers:
            seg_gathers.setdefault((g["sb"], g["bank"]), []).append(g)
        g_slots = {}
        for sl in ls.slots:
            g_slots.setdefault(sl["g"], []).append(sl)
        return seg_gathers, g_slots

    seg1, gsl1 = layer_maps(sch.L1)
    seg2, gsl2 = layer_maps(sch.L2)

    ag2_at_sb = {}
    for q in range(4):
        sq_end = (qb[q][1] - 1) // SB_N
        key = min(sq_end + AG2_LAG, n_sb - 1) if q < 3 else n_sb - 1
        ag2_at_sb.setdefault(key, []).append(q)

    # layer-2 gathers pre-issued before the final AG2 trigger (the trigger
    # blocks the gpsimd SEQ until the collective completes)
    l2_pre = [(0, 0), (1, 0), (0, 1), (1, 1), (0, 2), (1, 2)]

    with tile.TileContext(nc) as tc:
        with (
            tc.tile_pool(name="const", bufs=1) as cpool,
            tc.tile_pool(name="xload", bufs=2) as xpool,
            tc.tile_pool(name="hb", bufs=3) as hbpool,
            tc.tile_pool(name="idx", bufs=12) as ipool,
            tc.tile_pool(name="gath", bufs=GBUFS) as gpool,
            tc.tile_pool(name="sind", bufs=SBUFS) as spool,
            tc.tile_pool(name="dvp", bufs=12) as dvpool,
            tc.tile_pool(name="rl", bufs=12) as rpool,
            tc.tile_pool(name="blk", bufs=3) as bpool,
            tc.tile_pool(name="psh", bufs=2, space="PSUM") as psh,
            tc.tile_pool(name="psagg", bufs=SB_N, space="PSUM") as psagg,
            tc.tile_pool(name="psh2", bufs=1, space="PSUM") as psh2,
        ):
            import contextlib

            regstack = contextlib.ExitStack()
            nidx_vals = sorted(
                {g["nidx"] for g in sch.L1.gathers}
                | {g["nidx"] for g in sch.L2.gathers}
            )
            nreg_map = {}
            for v in nidx_vals:
                r = regstack.enter_context(nc.gpsimd.register(f"nreg_{v}"))
                nc.gpsimd.reg_mov(r, v)
                nreg_map[v] = r

            # ---- constants into SBUF
            w1_t = [
                cpool.tile([P, hid], bf16, tag=f"w1_{k}", name=f"w1t{k}")
                for k in range(kin)
            ]
            for k in range(kin):
                nc.sync.dma_start(out=w1_t[k][:], in_=w1[k * P : (k + 1) * P, :])
            w2_sb = cpool.tile([hid, out_dim], bf16, tag="w2")
            nc.sync.dma_start(out=w2_sb[:], in_=w2[:])
            b1_sb = cpool.tile([hid, 1], f32, tag="b1")
            nc.sync.dma_start(out=b1_sb[:], in_=b1[:])
            b2_sb = cpool.tile([P, out_dim], f32, tag="b2")
            nc.sync.dma_start(out=b2_sb[:], in_=b2bc[:])
            dinvb_sb = cpool.tile([P, nblk], f32, tag="dinvb")
            nc.sync.dma_start(out=dinvb_sb[:], in_=dinvb[:])
            dstloc1_sb = cpool.tile([P, sch.L1.nslots], bf16, tag="dstloc1")
            nc.sync.dma_start(out=dstloc1_sb[:], in_=dstloc1[:])
            dstloc2_sb = cpool.tile([P, sch.L2.nslots], bf16, tag="dstloc2")
            nc.sync.dma_start(out=dstloc2_sb[:], in_=dstloc2[:])
            iotar_sb = cpool.tile([P, slotmax * P], bf16, tag="iotar")
            nc.sync.dma_start(out=iotar_sb[:], in_=iotar_in[:])
            ident_sb = cpool.tile([P, P], bf16, tag="ident")
            nc.sync.dma_start(out=ident_sb[:], in_=ident_in[:])

            # ---- h~ = (dinv*x) @ W1, shard-local (x pre-scaled by dinv);
            # 4 blocks batched per PSUM bank, double-buffered
            for g0 in range(0, nblk, XGRP):
                g1 = min(g0 + XGRP, nblk)
                c0, c1 = g0 * P, min(g1 * P, shard)
                xt = [
                    xpool.tile([P, XGRP * P], bf16, tag=f"xt{k}", name=f"xt{k}")
                    for k in range(kin)
                ]
                for k in range(kin):
                    nc.sync.dma_start(
                        out=xt[k][:, : c1 - c0], in_=xT[k * P : (k + 1) * P, c0:c1]
                    )
                for s0 in range(g0, g1, 4):
                    s1 = min(s0 + 4, g1)
                    hp = psh.tile([P, 4 * hid], f32, tag="hps")
                    mtot = min(s1 * P, shard) - s0 * P
                    for b in range(s0, s1):
                        m = min(P, shard - b * P)
                        sub = b - s0
                        for k in range(kin):
                            nc.tensor.matmul(
                                out=hp[:m, sub * hid : (sub + 1) * hid],
                                lhsT=xt[k][:, b * P - c0 : b * P - c0 + m],
                                rhs=w1_t[k][:],
                                start=(k == 0),
                                stop=(k == kin - 1),
                            )
                    nsub = s1 - s0
                    hsb = hbpool.tile([P, 4, hid], bf16, tag="hsb")
                    nc.scalar.activation(
                        out=hsb[:, :nsub, :],
                        in_=hp[:, : nsub * hid].rearrange("p (g f) -> p g f", g=nsub),
                        func=mybir.ActivationFunctionType.Copy,
                    )
                    nfull = mtot // P
                    if nfull:
                        nc.sync.dma_start(
                            out=hloc[s0 * P : s0 * P + nfull * P, :].rearrange(
                                "(g p) f -> p g f", p=P
                            ),
                            in_=hsb[:, :nfull, :],
                        )
                    if mtot > nfull * P:
                        mp = mtot - nfull * P
                        nc.sync.dma_start(
                            out=hloc[s0 * P + nfull * P : s0 * P + mtot, :],
                            in_=hsb[:mp, nfull, :],
                        )

            nc.gpsimd.collective_compute(
                "AllGather",
                mybir.AluOpType.bypass,
                ins=[hloc[:]],
                outs=[hfull[:]],
                replica_groups=[list(range(NCORES))],
            )

            def bank_table(layer, k):
                if layer == 1:
                    r0 = k * sch.bank1_rows
                    r1 = min(r0 + sch.bank1_rows, n)
                    return hfull[r0:r1, :]
                return h2bank[k][0 : sch.bank2_n[k], :]

            def issue_gather(layer, g, it):
                nidx, nch = g["nidx"], g["nch"]
                gt = gpool.tile([P, gchmax, P], bf16, tag="gt")
                nc.gpsimd.dma_gather(
                    out_ap=gt[:, :nch, :],
                    in_ap=bank_table(layer, g["bank"]),
                    idxs_ap=it[:, : nidx // 16],
                    num_idxs=nidx,
                    num_idxs_reg=nreg_map[nidx],
                    elem_size=P,
                    single_packet=False,
                    queue_num=g["bank"],
                )
                return gt

            def load_it(layer, g):
                idxs = idxs1 if layer == 1 else idxs2
                it = ipool.tile([P, gchmax * 8], mybir.dt.int16, tag="it")
                nc.sync.dma_start(
                    out=it[:, : g["nidx"] // 16],
                    in_=idxs[:, g["c16"] : g["c16"] + g["nidx"] // 16],
                )
                return it

            pre_issued = {}  # (layer, gi) -> gt tile

            def run_layer(layer):
                seg_gathers = seg1 if layer == 1 else seg2
                g_slots = gsl1 if layer == 1 else gsl2
                ls = sch.L1 if layer == 1 else sch.L2
                dstloc_sb = dstloc1_sb if layer == 1 else dstloc2_sb
                w = P if layer == 1 else out_dim

                def prefetch_sb(s):
                    tiles = {}
                    if s >= n_sb:
                        return tiles
                    for k in range(4):
                        for g in seg_gathers.get((s, k), []):
                            if (layer, g["gi"]) in pre_issued:
                                continue
                            tiles[g["gi"]] = load_it(layer, g)
                    return tiles

                def prefetch_ind(s):
                    tiles = {}
                    if s >= n_sb:
                        return tiles
                    for k in range(4):
                        for g in seg_gathers.get((s, k), []):
                            nsl = g["nslots"]
                            sbig = spool.tile([P, slotmax, P], bf16, tag="sind")
                            nc.vector.tensor_tensor(
                                out=sbig[:, :nsl, :],
                                in0=iotar_sb[:, : nsl * P].rearrange(
                                    "p (k f) -> p k f", k=nsl
                                ),
                                in1=dstloc_sb[
                                    :, g["slot0"] : g["slot0"] + nsl
                                ].to_broadcast([P, nsl, P]),
                                op=mybir.AluOpType.is_equal,
                            )
                            tiles[g["gi"]] = sbig
                    return tiles

                def prefetch_rl(s):
                    tiles = {}
                    if s >= n_sb:
                        return tiles
                    for b in range(s * SB_N, min((s + 1) * SB_N, nblk)):
                        m = min(P, shard - b * P)
                        rl = rpool.tile([P, P], bf16, tag="rl")
                        if layer == 1:
                            nc.sync.dma_start(
                                out=rl[:m, :], in_=hloc[b * P : b * P + m, :]
                            )
                        else:
                            q = quarter_of(b)
                            r0 = b * P - sch.qrow_start[q]
                            nc.sync.dma_start(
                                out=rl[:m, :], in_=h2loc_q[q][r0 : r0 + m, :]
                            )
                        tiles[b] = rl
                    return tiles

                it_tiles = prefetch_sb(0)
                rl_tiles = prefetch_rl(0)
                ind_tiles = prefetch_ind(0)
                for s in range(n_sb):
                    blocks = list(range(s * SB_N, min((s + 1) * SB_N, nblk)))
                    next_tiles = prefetch_sb(s + 1)
                    next_ind = prefetch_ind(s + 1)
                    next_rl = prefetch_rl(s + 1)
                    dv_tiles = {}
                    if layer == 1:
                        for b in blocks:
                            dv = dvpool.tile([P, P], f32, tag="dv")
                            nc.sync.dma_start(
                                out=dv[:], in_=dinvbc[:, b * P : (b + 1) * P]
                            )
                            dv_tiles[b] = dv
                    agg_t = {
                        b: psagg.tile([P, w], f32, tag="agg", name=f"agg{layer}_{s}_{b}")
                        for b in blocks
                    }
                    # self-loop contribution opens each block's PSUM group
                    for b in blocks:
                        m = min(P, shard - b * P)
                        rl = rl_tiles[b]
                        solo = (s, b) not in ls.has_slots
                        if layer == 1:
                            nc.tensor.matmul(
                                out=agg_t[b][:, :],
                                lhsT=rl[:m, :],
                                rhs=ident_sb[:m, :],
                                start=True,
                                stop=solo,
                            )
                        else:
                            nc.tensor.matmul(
                                out=agg_t[b][:, :],
                                lhsT=ident_sb[:m, :],
                                rhs=rl[:m, :out_dim],
                                start=True,
                                stop=solo,
                            )
                    for k in range(4):
                        for g in seg_gathers.get((s, k), []):
                            if (layer, g["gi"]) in pre_issued:
                                gt = pre_issued.pop((layer, g["gi"]))
                            else:
                                gt = issue_gather(layer, g, it_tiles[g["gi"]])
                            sbig = ind_tiles[g["gi"]]
                            for sl in g_slots.get(g["gi"], []):
                                if layer == 1:
                                    nc.tensor.matmul(
                                        out=agg_t[sl["blk"]][:, :],
                                        lhsT=gt[:, sl["cl"], :],
                                        rhs=sbig[:, sl["sl_local"], :],
                                        start=sl["start"],
                                        stop=sl["stop"],
                                    )
                                else:
                                    nc.tensor.matmul(
                                        out=agg_t[sl["blk"]][:, :],
                                        lhsT=sbig[:, sl["sl_local"], :],
                                        rhs=gt[:, sl["cl"], :out_dim],
                                        start=sl["start"],
                                        stop=sl["stop"],
                                    )
                    # ---- block epilogues
                    for b in blocks:
                        m = min(P, shard - b * P)
                        if layer == 1:
                            t1 = bpool.tile([P, P], bf16, tag="t1")
                            nc.vector.tensor_tensor(
                                out=t1[:],
                                in0=agg_t[b][:, :],
                                in1=dv_tiles[b][:],
                                op=mybir.AluOpType.mult,
                            )
                            o1 = bpool.tile([P, P], bf16, tag="o1")
                            nc.scalar.activation(
                                out=o1[:],
                                in_=t1[:],
                                func=mybir.ActivationFunctionType.Relu,
                                bias=b1_sb[:, :1],
                            )
                            h2p = psh2.tile([P, out_dim], f32, tag="h2p")
                            nc.tensor.matmul(
                                out=h2p[:],
                                lhsT=o1[:],
                                rhs=w2_sb[:],
                                start=True,
                                stop=True,
                            )
                            h2s = bpool.tile([P, P], bf16, tag="h2s")
                            nc.scalar.activation(
                                out=h2s[:m, :out_dim],
                                in_=h2p[:m, :],
                                func=mybir.ActivationFunctionType.Copy,
                                scale=dinvb_sb[:m, b : b + 1],
                            )
                            q = quarter_of(b)
                            r0 = b * P - sch.qrow_start[q]
                            nc.sync.dma_start(
                                out=h2loc_q[q][r0 : r0 + m, 0:out_dim],
                                in_=h2s[:m, :out_dim],
                            )
                        else:
                            t2 = bpool.tile([P, out_dim], f32, tag="t2")
                            nc.scalar.activation(
                                out=t2[:m, :],
                                in_=agg_t[b][:m, :],
                                func=mybir.ActivationFunctionType.Copy,
                                scale=dinvb_sb[:m, b : b + 1],
                            )
                            ob = bpool.tile([P, out_dim], f32, tag="ob")
                            nc.vector.tensor_tensor(
                                out=ob[:m, :],
                                in0=t2[:m, :],
                                in1=b2_sb[:m, :],
                                op=mybir.AluOpType.add,
                            )
                            nc.sync.dma_start(
                                out=out_ext[b * P : b * P + m, :], in_=ob[:m, :]
                            )
                    rl_tiles = next_rl
                    ind_tiles = next_ind
                    if layer == 1:
                        trig = ag2_at_sb.get(s, [])
                        if 3 in trig:
                            # pre-issue layer-2 bank-0..2 gathers so the Q7
                            # queues stay busy while the final AG2 runs
                            for (s2, k2) in l2_pre:
                                for g in seg2.get((s2, k2), []):
                                    it = load_it(2, g)
                                    pre_issued[(2, g["gi"])] = issue_gather(2, g, it)
                        for q in trig:
                            nc.gpsimd.collective_compute(
                                "AllGather",
                                mybir.AluOpType.bypass,
                                ins=[h2loc_q[q][:]],
                                outs=[h2bank[q][:]],
                                replica_groups=[list(range(NCORES))],
                            )
                    it_tiles = next_tiles

            run_layer(1)
            run_layer(2)
            regstack.close()

    nc.compile()
    return nc


# ---------------------------------------------------------------- kernel ---
def _make_in_maps(sch, x, W1, b1v, W2, b2v):
    hid = W1.shape[1]
    out_dim = W2.shape[1]
    shard, nblk = sch.shard, sch.nblk
    bf = ml_dtypes.bfloat16
    in_maps = []
    w1b = W1.astype(bf)
    w2b = W2.astype(bf)
    b1c = b1v.reshape(hid, 1).astype(np.float32).copy()
    b2c = np.broadcast_to(b2v.astype(np.float32), (P, out_dim)).copy()
    iotar = np.tile(np.arange(P, dtype=np.float32), (P, sch.slotmax)).astype(bf)
    ident = np.eye(P, dtype=np.float32).astype(bf)
    xs_all = (x * sch.dinv[:, None]).astype(bf)
    for c in range(NCORES):
        xs = np.ascontiguousarray(xs_all[c * shard : (c + 1) * shard].T)
        dv = sch.dinv[c * shard : (c + 1) * shard]
        full = np.zeros(nblk * P, np.float32)
        full[:shard] = dv
        dvb = np.ascontiguousarray(full.reshape(nblk, P).T)
        dbc = np.broadcast_to(full, (P, nblk * P)).copy()
        in_maps.append(
            {
                "xT": xs,
                "idxs1": sch.L1.idx_stream[c],
                "idxs2": sch.L2.idx_stream[c],
                "dstloc1": sch.L1.dstloc_s[c],
                "dstloc2": sch.L2.dstloc_s[c],
                "dinvb": dvb,
                "W1": w1b,
                "b1": b1c,
                "W2": w2b,
                "b2bc": b2c,
                "iotar": iotar,
                "ident": ident,
                "dinvbc": dbc,
            }
        )
    return in_maps


def _get_compiled(n, e, edge_index, in_dim, hid, out_dim):
    key = ("nc", n, e)
    if key not in _CACHE:
        sch = _preprocess(n, edge_index)
        _CACHE[("sched", n, e)] = sch
        _CACHE[key] = _build(sch, in_dim, hid, out_dim)
    return _CACHE[("sched", n, e)], _CACHE[key]


def kernel(x, edge_index, W1, b1, W2, b2):
    _install_compat()
    from concourse.bass_utils import run_bass_kernel_spmd

    x = np.asarray(x)
    edge_index = np.asarray(edge_index)
    W1 = np.asarray(W1, np.float32)
    b1v = np.asarray(b1, np.float32)
    W2 = np.asarray(W2, np.float32)
    b2v = np.asarray(b2, np.float32)
    n, in_dim = x.shape
    hid = W1.shape[1]
    out_dim = W2.shape[1]

    sch, nc = _get_compiled(n, edge_index.shape[1], edge_index, in_dim, hid, out_dim)
    in_maps = _make_in_maps(sch, x, W1, b1v, W2, b2v)
    import os

    trace = bool(os.environ.get("GCN_TRACE"))
    res = run_bass_kernel_spmd(
        nc, in_maps, core_ids=list(range(NCORES)), trace=trace
    )
    global LAST_EXEC_NS
    LAST_EXEC_NS = res.exec_time_ns
    return np.concatenate([res.results[c]["out"] for c in range(NCORES)], axis=0)


LAST_EXEC_NS = None


# revision 37
# speedup vs baseline: 1.7559x; 1.0920x over previous
"""2-layer GCN (GCNConv x2) on 8 Trainium2 NeuronCores.

Strategy (dst-sharded, edge-partitioned by destination; Q7-descgen-optimized):
- Each core owns N/8 destination nodes and the edges pointing at them
  (plus the GCN self-loops, kept in the edge stream).
- Layer-1 table: h~ = (dinv*x) @ W1 computed per-shard (x host-prescaled by
  dinv), ONE AllGather into hfull; gather banks = 4 int16-addressable row
  slices of hfull (rank-pair layout). A single collective minimizes the
  ~60-95us per-op CC-stream overhead of this stack.
- Layer-2 table: h2~ = dinv * h2 written per-quarter (block-aligned
  quarters), 4 bank-wise AllGathers fired as quarters complete so they
  overlap layer-1's gather phase; bank q = concat over cores of quarter q.
- Since the two layers use different bank layouts, each layer has its own
  edge schedule (idx stream / dstloc / slot table); the dst-side blocking
  is shared.
- Per (super-block of 5 dst blocks, bank): edges packed contiguously in
  block order into 128-slot chunks (straddling block boundaries); one
  dma_gather per segment on queue=bank (4 SWDGE queue contexts, balanced).
  Scatter-add via is_equal-indicator matmuls accumulating in PSUM.
- idx tiles and epilogue dv tiles prefetched one super-block ahead so the
  sync-engine FIFO (epilogue stores) never delays gather dispatch; layer 2's
  first bank-0..2 gathers are pre-issued BEFORE the last AG2 trigger
  (collective triggers block the gpsimd SEQ until completion).
- Layer 1 accumulates transposed (aggT [feat, dst]) so bias+ReLU ride the
  activation engine and out1 feeds h2 = out1 @ W2 directly as lhsT.
"""
import sys
import types

import numpy as np
import ml_dtypes

P = 128
NCORES = 8
GMAX = 32  # max chunks (128 idxs each) per dma_gather
SB_N = 3  # dst blocks per super-block; psagg 2*SB_N banks -> 2 sbs in flight
NQUEUES = 4
XGRP = 8
GBUFS = 14
SBUFS = 10
AG2_LAG = 2  # super-blocks between a quarter finishing and its AG2 trigger

_CACHE = {}


# ---------------------------------------------------------------- compat ---
def _install_compat():
    """Patches for this axon/walrus stack (drain waits, per-inst wait caps,
    NTFF shim). Idempotent."""
    if _CACHE.get("compat"):
        return
    import concourse.tile as tile
    import concourse.mybir as mybir

    _ev = [0]

    def _split_inst_waits(ordered):
        for _bb, insts in ordered.items():
            out = []
            for inst in insts:
                si = getattr(inst, "sync_info", None)
                if si is not None and si.on_wait is not None and len(si.on_wait) > 1:
                    waits = list(si.on_wait)
                    excess, keep = waits[:-1], waits[-1:]
                    si.on_wait.clear()
                    for sw in keep:
                        si.on_wait.append(sw)
                    for i in range(0, len(excess), 2):
                        _ev[0] += 1
                        ev = mybir.InstEventSemaphore(
                            name=f"evsplit-{_ev[0]}", ins=[], outs=[]
                        )
                        ev.engine = inst.engine
                        ev.sync_info = mybir.SyncInfo(
                            on_wait=excess[i : i + 2], on_update=[]
                        )
                        out.append(ev)
                out.append(inst)
            insts[:] = out

    orig_lower = tile.TileContext._lower_ordered_insts

    def patched_lower(self, ordered):
        _split_inst_waits(ordered)
        return orig_lower(self, ordered)

    def patched_drain(self, tick_clock, wait_clock):
        sems_alloc = list(self.sems.allocated().values())
        carrier = self.nc.sync.wait_ge(sems_alloc[0], 0)
        wait_clock.add_sem_waits(
            carrier.ins, tile.ScopedClock({None: tick_clock.global_clock})
        )
        waits = list(carrier.ins.sync_info.on_wait)
        carrier.ins.sync_info.on_wait.clear()
        for sw in waits[:2]:
            carrier.ins.sync_info.on_wait.append(sw)
        for i in range(2, len(waits), 2):
            c = self.nc.sync.wait_ge(sems_alloc[0], 0)
            c.ins.sync_info.on_wait.clear()
            for sw in waits[i : i + 2]:
                c.ins.sync_info.on_wait.append(sw)
        self.nc.sync.drain(fusable=False)
        self.nc.all_engine_barrier()
        popped = self.nc._tile_sem_poison_stack.pop()
        assert popped is self._sem_poison
        self.nc.clear_and_free_semaphores(sems_alloc)
        self.nc.all_engine_barrier()

    tile.TileContext._lower_ordered_insts = patched_lower
    tile.TileContext._drain_and_barrier = patched_drain

    # NTFF profile hook shim (missing antenv.axon_hooks in this image)
    _hook = {}
    mod = types.ModuleType("antenv.axon_hooks")
    mod.set_axon_ntff_profile_hook = lambda h: _hook.update(hook=h)
    mod.get_axon_ntff_profile_hook = lambda: _hook.get("hook")
    sys.modules["antenv.axon_hooks"] = mod
    try:
        import antenv

        antenv.axon_hooks = mod
        from trn_agent_boot.trn_boot import _ntff_profile_via_ctypes

        mod.set_axon_ntff_profile_hook(
            _ntff_profile_via_ctypes("/opt/axon/libaxon_pjrt.so")
        )
    except Exception:
        pass
    _CACHE["compat"] = True


# ---------------------------------------------------------- preprocessing ---
class Schedule:
    pass


class LayerSched:
    pass


def _quarter_bounds(nblk):
    base, rem = nblk // 4, nblk % 4
    sizes = [base + (1 if i < rem else 0) for i in range(4)]
    starts = np.cumsum([0] + sizes)
    return [(int(starts[i]), int(starts[i + 1])) for i in range(4)]


def _make_layer_sched(
    n, nblk, n_sb, s_core, s_sb, s_bank, s_blk, s_bidx, s_dstloc
):
    """Build the per-(super-block, bank) straddle-packed schedule for one
    bank mapping. Inputs are edge arrays sorted by (core, sb, bank, blk)."""
    e = s_core.shape[0]
    key = ((s_core * n_sb + s_sb) * 4 + s_bank) * nblk + s_blk
    cnt = np.bincount(key, minlength=NCORES * n_sb * 4 * nblk).reshape(
        NCORES, n_sb, 4, nblk
    )

    gathers = []
    slots = []
    first_slot_of_block = {}
    last_slot_of_block = {}
    chunk_gid = 0
    slot_gid = 0
    budget_tab = np.zeros((n_sb, 4), np.int64)
    for s in range(n_sb):
        blocks = list(range(s * SB_N, min((s + 1) * SB_N, nblk)))
        for k in range(4):
            percore = cnt[:, s, k, :][:, blocks]
            cum = np.cumsum(percore, axis=1)
            budget = max(int(np.ceil(cum[:, -1].max() / P)), 1)
            budget_tab[s, k] = budget
            lo = np.min(cum - percore, axis=0)
            hi = np.max(cum, axis=0)
            g0 = 0
            while g0 < budget:
                gn = min(GMAX, budget - g0)
                gi = len(gathers)
                gslot0 = slot_gid
                for j in range(g0, g0 + gn):
                    c_lo, c_hi = j * P, (j + 1) * P
                    for bi, b in enumerate(blocks):
                        if hi[bi] > c_lo and lo[bi] < c_hi:
                            slots.append(
                                dict(
                                    g=gi,
                                    cl=j - g0,
                                    blk=b,
                                    sb=s,
                                    bank=k,
                                    sl_local=slot_gid - gslot0,
                                    chunk_gid=chunk_gid + j,
                                )
                            )
                            first_slot_of_block.setdefault((s, b), slot_gid)
                            last_slot_of_block[(s, b)] = slot_gid
                            slot_gid += 1
                gathers.append(
                    dict(
                        gi=gi,
                        sb=s,
                        bank=k,
                        chunk0=chunk_gid + g0,
                        nch=gn,
                        nidx=gn * P,
                        slot0=gslot0,
                        nslots=slot_gid - gslot0,
                        c16=(chunk_gid + g0) * P // 16,
                    )
                )
                g0 += gn
            chunk_gid += budget
    totc = chunk_gid
    nslots = slot_gid
    tot_slots = totc * P
    # the self-loop identity matmul opens each block's PSUM group (start);
    # the last slot closes it (stop)
    for i, sl in enumerate(slots):
        sl["start"] = False
        sl["stop"] = last_slot_of_block[(sl["sb"], sl["blk"])] == i
    has_slots = set(first_slot_of_block.keys())

    seg_key = (s_core * n_sb + s_sb) * 4 + s_bank
    seg_ptr = np.searchsorted(seg_key, np.arange(NCORES * n_sb * 4 + 1))
    seg_chunk0 = {}
    cg = 0
    for s in range(n_sb):
        for k in range(4):
            seg_chunk0[(s, k)] = cg
            cg += int(budget_tab[s, k])

    idx_flat = np.zeros((NCORES, tot_slots), np.int16)
    for c in range(NCORES):
        arr = idx_flat[c]
        for s in range(n_sb):
            for k in range(4):
                p0 = seg_ptr[(c * n_sb + s) * 4 + k]
                p1 = seg_ptr[(c * n_sb + s) * 4 + k + 1]
                base = seg_chunk0[(s, k)] * P
                arr[base : base + (p1 - p0)] = s_bidx[p0:p1].astype(np.int16)

    dstloc_s = np.full((NCORES, P, nslots), -1.0, np.float32)
    seg_id = (s_core * n_sb + s_sb) * 4 + s_bank
    pos_in_seg = np.arange(e) - seg_ptr[seg_id]
    seg_chunk0_arr = np.zeros(NCORES * n_sb * 4, np.int64)
    for s in range(n_sb):
        for k in range(4):
            for c in range(NCORES):
                seg_chunk0_arr[(c * n_sb + s) * 4 + k] = seg_chunk0[(s, k)]
    chunk_of_edge = seg_chunk0_arr[seg_id] + pos_in_seg // P
    part_of_edge = pos_in_seg % P
    slot_lut = np.full((totc, nblk), -1, np.int64)
    for i, sl in enumerate(slots):
        slot_lut[sl["chunk_gid"], sl["blk"]] = i
    slot_of_edge = slot_lut[chunk_of_edge, s_blk]
    assert (slot_of_edge >= 0).all()
    dstloc_s[s_core, part_of_edge, slot_of_edge] = s_dstloc

    idx_stream = np.ascontiguousarray(
        idx_flat.reshape(NCORES, tot_slots // 16, 16).transpose(0, 2, 1)
    )
    idx_stream = np.tile(idx_stream, (1, 8, 1))

    ls = LayerSched()
    ls.has_slots = has_slots
    ls.gathers = gathers
    ls.slots = slots
    ls.totc, ls.nslots, ls.tot_slots = totc, nslots, tot_slots
    ls.slotmax = max(g["nslots"] for g in gathers)
    ls.gchmax = max(g["nch"] for g in gathers)
    ls.idx_stream = idx_stream
    ls.idx_flat = idx_flat
    ls.dstloc_s = dstloc_s.astype(ml_dtypes.bfloat16)
    return ls


def _preprocess(n, edge_index):
    src0 = np.asarray(edge_index[0], np.int64)
    dst0 = np.asarray(edge_index[1], np.int64)
    shard = n // NCORES
    nblk = (shard + P - 1) // P
    n_sb = (nblk + SB_N - 1) // SB_N
    qb = _quarter_bounds(nblk)
    qrow_start = [b0 * P for b0, b1 in qb]
    qrows = [min(b1 * P, shard) - b0 * P for b0, b1 in qb]
    bank2_n = [NCORES * r for r in qrows]
    assert all(b <= 32767 for b in bank2_n)
    bank1_rows = (n + 3) // 4  # rank-pair banks for layer 1 (slices of hfull)
    assert bank1_rows <= 32767

    deg = np.bincount(dst0, minlength=n).astype(np.float64) + 1.0
    dinv = (1.0 / np.sqrt(deg)).astype(np.float32)

    # self-loops are NOT in the edge stream: both layers fold them in with
    # one identity matmul per block from the per-core hloc/h2loc tensors.
    src = src0
    dst = dst0

    core_d = dst // shard
    dl = dst - core_d * shard
    blk = dl // P
    dstloc = (dl % P).astype(np.int64)
    sb = blk // SB_N

    # layer-1 bank mapping: contiguous row slices of hfull (rank-major)
    bank_a = src // bank1_rows
    bidx_a = src - bank_a * bank1_rows

    # layer-2 bank mapping: quarter-stacked
    core_s = src // shard
    off = src - core_s * shard
    sblk = off // P
    qb_arr = np.zeros(nblk, np.int64)
    for q, (b0, b1) in enumerate(qb):
        qb_arr[b0:b1] = q
    bank_b = qb_arr[sblk]
    bidx_b = core_s * np.array(qrows)[bank_b] + (off - np.array(qrow_start)[bank_b])

    scheds = []
    for bank, bidx in ((bank_a, bidx_a), (bank_b, bidx_b)):
        order = np.lexsort((blk, bank, sb, core_d))
        scheds.append(
            _make_layer_sched(
                n,
                nblk,
                n_sb,
                core_d[order],
                sb[order],
                bank[order],
                blk[order],
                bidx[order],
                dstloc[order],
            )
        )

    sch = Schedule()
    sch.n, sch.shard, sch.nblk, sch.n_sb = n, shard, nblk, n_sb
    sch.e = src.shape[0]
    sch.qb, sch.qrow_start, sch.qrows = qb, qrow_start, qrows
    sch.bank1_rows, sch.bank2_n = bank1_rows, bank2_n
    sch.dinv = dinv
    sch.L1, sch.L2 = scheds
    sch.slotmax = max(sch.L1.slotmax, sch.L2.slotmax)
    return sch


# ----------------------------------------------------------------- build ---
def _build(sch, in_dim, hid, out_dim):
    import concourse.mybir as mybir
    import concourse.tile as tile
    from concourse import bacc

    bf16 = mybir.dt.bfloat16
    f32 = mybir.dt.float32
    shard, nblk, n_sb = sch.shard, sch.nblk, sch.n_sb
    slotmax = sch.slotmax
    gchmax = max(sch.L1.gchmax, sch.L2.gchmax)
    qb = sch.qb
    n = sch.n

    nc = bacc.Bacc(num_swdge_queues=NQUEUES)

    xT = nc.declare_dram_parameter("xT", [in_dim, shard], bf16, isOutput=False)
    idxs1 = nc.declare_dram_parameter(
        "idxs1", [P, sch.L1.tot_slots // 16], mybir.dt.int16, isOutput=False
    )
    idxs2 = nc.declare_dram_parameter(
        "idxs2", [P, sch.L2.tot_slots // 16], mybir.dt.int16, isOutput=False
    )
    dstloc1 = nc.declare_dram_parameter(
        "dstloc1", [P, sch.L1.nslots], bf16, isOutput=False
    )
    dstloc2 = nc.declare_dram_parameter(
        "dstloc2", [P, sch.L2.nslots], bf16, isOutput=False
    )
    iotar_in = nc.declare_dram_parameter("iotar", [P, slotmax * P], bf16, isOutput=False)
    dinvbc = nc.declare_dram_parameter("dinvbc", [P, nblk * P], f32, isOutput=False)
    dinvb = nc.declare_dram_parameter("dinvb", [P, nblk], f32, isOutput=False)
    w1 = nc.declare_dram_parameter("W1", [in_dim, hid], bf16, isOutput=False)
    b1 = nc.declare_dram_parameter("b1", [hid, 1], f32, isOutput=False)
    w2 = nc.declare_dram_parameter("W2", [hid, out_dim], bf16, isOutput=False)
    b2bc = nc.declare_dram_parameter("b2bc", [P, out_dim], f32, isOutput=False)
    ident_in = nc.declare_dram_parameter("ident", [P, P], bf16, isOutput=False)
    out_ext = nc.declare_dram_parameter("out", [shard, out_dim], f32, isOutput=True)

    hloc = nc.dram_tensor("hloc", [shard, P], bf16)
    hfull = nc.dram_tensor("hfull", [n, P], bf16, addr_space="Shared")
    h2loc_q = [
        nc.dram_tensor(f"h2loc{q}", [sch.qrows[q], P], bf16) for q in range(4)
    ]
    h2bank = [
        nc.dram_tensor(f"h2bank{q}", [sch.bank2_n[q], P], bf16, addr_space="Shared")
        for q in range(4)
    ]

    kin = in_dim // P

    def quarter_of(b):
        for q, (b0, b1) in enumerate(qb):
            if b0 <= b < b1:
                return q
        raise AssertionError

    def layer_maps(ls):
        seg_gathers = {}
        for g in ls.gathers:
            seg_gathers.setdefault((g["sb"], g["bank"]), []).append(g)
        g_slots = {}
        for sl in ls.slots:
            g_slots.setdefault(sl["g"], []).append(sl)
        return seg_gathers, g_slots

    seg1, gsl1 = layer_maps(sch.L1)
    seg2, gsl2 = layer_maps(sch.L2)

    ag2_at_sb = {}
    for q in range(4):
        sq_end = (qb[q][1] - 1) // SB_N
        key = min(sq_end + AG2_LAG, n_sb - 1) if q < 3 else n_sb - 1
        ag2_at_sb.setdefault(key, []).append(q)

    # layer-2 gathers pre-issued before the final AG2 trigger (the trigger
    # blocks the gpsimd SEQ until the collective completes)
    l2_pre = [(s2, k2) for s2 in range(4) for k2 in range(3)]

    with tile.TileContext(nc) as tc:
        with (
            tc.tile_pool(name="const", bufs=1) as cpool,
            tc.tile_pool(name="xload", bufs=2) as xpool,
            tc.tile_pool(name="hb", bufs=3) as hbpool,
            tc.tile_pool(name="idx", bufs=16) as ipool,
            tc.tile_pool(name="gath", bufs=GBUFS) as gpool,
            tc.tile_pool(name="sind", bufs=SBUFS) as spool,
            tc.tile_pool(name="dvp", bufs=12) as dvpool,
            tc.tile_pool(name="rl", bufs=12) as rpool,
            tc.tile_pool(name="blk", bufs=3) as bpool,
            tc.tile_pool(name="psh", bufs=1, space="PSUM") as psh,
            tc.tile_pool(name="psagg", bufs=2 * SB_N, space="PSUM") as psagg,
            tc.tile_pool(name="psh2", bufs=1, space="PSUM") as psh2,
        ):
            import contextlib

            regstack = contextlib.ExitStack()
            nidx_vals = sorted(
                {g["nidx"] for g in sch.L1.gathers}
                | {g["nidx"] for g in sch.L2.gathers}
            )
            nreg_map = {}
            for v in nidx_vals:
                r = regstack.enter_context(nc.gpsimd.register(f"nreg_{v}"))
                nc.gpsimd.reg_mov(r, v)
                nreg_map[v] = r

            # ---- constants into SBUF
            w1_t = [
                cpool.tile([P, hid], bf16, tag=f"w1_{k}", name=f"w1t{k}")
                for k in range(kin)
            ]
            for k in range(kin):
                nc.sync.dma_start(out=w1_t[k][:], in_=w1[k * P : (k + 1) * P, :])
            w2_sb = cpool.tile([hid, out_dim], bf16, tag="w2")
            nc.sync.dma_start(out=w2_sb[:], in_=w2[:])
            b1_sb = cpool.tile([hid, 1], f32, tag="b1")
            nc.sync.dma_start(out=b1_sb[:], in_=b1[:])
            b2_sb = cpool.tile([P, out_dim], f32, tag="b2")
            nc.sync.dma_start(out=b2_sb[:], in_=b2bc[:])
            dinvb_sb = cpool.tile([P, nblk], f32, tag="dinvb")
            nc.sync.dma_start(out=dinvb_sb[:], in_=dinvb[:])
            dstloc1_sb = cpool.tile([P, sch.L1.nslots], bf16, tag="dstloc1")
            nc.sync.dma_start(out=dstloc1_sb[:], in_=dstloc1[:])
            dstloc2_sb = cpool.tile([P, sch.L2.nslots], bf16, tag="dstloc2")
            nc.sync.dma_start(out=dstloc2_sb[:], in_=dstloc2[:])
            iotar_sb = cpool.tile([P, slotmax * P], bf16, tag="iotar")
            nc.sync.dma_start(out=iotar_sb[:], in_=iotar_in[:])
            ident_sb = cpool.tile([P, P], bf16, tag="ident")
            nc.sync.dma_start(out=ident_sb[:], in_=ident_in[:])

            # ---- h~ = (dinv*x) @ W1, shard-local (x pre-scaled by dinv);
            # 4 blocks batched per PSUM bank, double-buffered
            for g0 in range(0, nblk, XGRP):
                g1 = min(g0 + XGRP, nblk)
                c0, c1 = g0 * P, min(g1 * P, shard)
                xt = [
                    xpool.tile([P, XGRP * P], bf16, tag=f"xt{k}", name=f"xt{k}")
                    for k in range(kin)
                ]
                for k in range(kin):
                    nc.sync.dma_start(
                        out=xt[k][:, : c1 - c0], in_=xT[k * P : (k + 1) * P, c0:c1]
                    )
                for s0 in range(g0, g1, 4):
                    s1 = min(s0 + 4, g1)
                    hp = psh.tile([P, 4 * hid], f32, tag="hps")
                    mtot = min(s1 * P, shard) - s0 * P
                    for b in range(s0, s1):
                        m = min(P, shard - b * P)
                        sub = b - s0
                        for k in range(kin):
                            nc.tensor.matmul(
                                out=hp[:m, sub * hid : (sub + 1) * hid],
                                lhsT=xt[k][:, b * P - c0 : b * P - c0 + m],
                                rhs=w1_t[k][:],
                                start=(k == 0),
                                stop=(k == kin - 1),
                            )
                    nsub = s1 - s0
                    hsb = hbpool.tile([P, 4, hid], bf16, tag="hsb")
                    nc.scalar.activation(
                        out=hsb[:, :nsub, :],
                        in_=hp[:, : nsub * hid].rearrange("p (g f) -> p g f", g=nsub),
                        func=mybir.ActivationFunctionType.Copy,
                    )
                    nfull = mtot // P
                    if nfull:
                        nc.sync.dma_start(
                            out=hloc[s0 * P : s0 * P + nfull * P, :].rearrange(
                                "(g p) f -> p g f", p=P
                            ),
                            in_=hsb[:, :nfull, :],
                        )
                    if mtot > nfull * P:
                        mp = mtot - nfull * P
                        nc.sync.dma_start(
                            out=hloc[s0 * P + nfull * P : s0 * P + mtot, :],
                            in_=hsb[:mp, nfull, :],
                        )

            nc.gpsimd.collective_compute(
                "AllGather",
                mybir.AluOpType.bypass,
                ins=[hloc[:]],
                outs=[hfull[:]],
                replica_groups=[list(range(NCORES))],
            )

            def bank_table(layer, k):
                if layer == 1:
                    r0 = k * sch.bank1_rows
                    r1 = min(r0 + sch.bank1_rows, n)
                    return hfull[r0:r1, :]
                return h2bank[k][0 : sch.bank2_n[k], :]

            def issue_gather(layer, g, it):
                nidx, nch = g["nidx"], g["nch"]
                gt = gpool.tile([P, gchmax, P], bf16, tag="gt")
                nc.gpsimd.dma_gather(
                    out_ap=gt[:, :nch, :],
                    in_ap=bank_table(layer, g["bank"]),
                    idxs_ap=it[:, : nidx // 16],
                    num_idxs=nidx,
                    num_idxs_reg=nreg_map[nidx],
                    elem_size=P,
                    single_packet=False,
                    queue_num=g["bank"],
                )
                return gt

            def load_it(layer, g):
                idxs = idxs1 if layer == 1 else idxs2
                it = ipool.tile([P, gchmax * 8], mybir.dt.int16, tag="it")
                nc.sync.dma_start(
                    out=it[:, : g["nidx"] // 16],
                    in_=idxs[:, g["c16"] : g["c16"] + g["nidx"] // 16],
                )
                return it

            pre_issued = {}  # (layer, gi) -> gt tile

            def run_layer(layer):
                seg_gathers = seg1 if layer == 1 else seg2
                g_slots = gsl1 if layer == 1 else gsl2
                ls = sch.L1 if layer == 1 else sch.L2
                dstloc_sb = dstloc1_sb if layer == 1 else dstloc2_sb
                w = P if layer == 1 else out_dim

                def prefetch_sb(s):
                    tiles = {}
                    if s >= n_sb:
                        return tiles
                    for k in range(4):
                        for g in seg_gathers.get((s, k), []):
                            if (layer, g["gi"]) in pre_issued:
                                continue
                            tiles[g["gi"]] = load_it(layer, g)
                    return tiles

                def prefetch_ind(s):
                    tiles = {}
                    if s >= n_sb:
                        return tiles
                    for k in range(4):
                        for g in seg_gathers.get((s, k), []):
                            nsl = g["nslots"]
                            sbig = spool.tile([P, slotmax, P], bf16, tag="sind")
                            nc.vector.tensor_tensor(
                                out=sbig[:, :nsl, :],
                                in0=iotar_sb[:, : nsl * P].rearrange(
                                    "p (k f) -> p k f", k=nsl
                                ),
                                in1=dstloc_sb[
                                    :, g["slot0"] : g["slot0"] + nsl
                                ].to_broadcast([P, nsl, P]),
                                op=mybir.AluOpType.is_equal,
                            )
                            tiles[g["gi"]] = sbig
                    return tiles

                def prefetch_rl(s):
                    tiles = {}
                    if s >= n_sb:
                        return tiles
                    for b in range(s * SB_N, min((s + 1) * SB_N, nblk)):
                        m = min(P, shard - b * P)
                        rl = rpool.tile([P, P], bf16, tag="rl")
                        if layer == 1:
                            nc.sync.dma_start(
                                out=rl[:m, :], in_=hloc[b * P : b * P + m, :]
                            )
                        else:
                            q = quarter_of(b)
                            r0 = b * P - sch.qrow_start[q]
                            nc.sync.dma_start(
                                out=rl[:m, :], in_=h2loc_q[q][r0 : r0 + m, :]
                            )
                        tiles[b] = rl
                    return tiles

                it_tiles = prefetch_sb(0)
                rl_tiles = prefetch_rl(0)
                ind_tiles = prefetch_ind(0)
                for s in range(n_sb):
                    blocks = list(range(s * SB_N, min((s + 1) * SB_N, nblk)))
                    next_tiles = prefetch_sb(s + 1)
                    next_ind = prefetch_ind(s + 1)
                    next_rl = prefetch_rl(s + 1)
                    dv_tiles = {}
                    if layer == 1:
                        for b in blocks:
                            dv = dvpool.tile([P, P], f32, tag="dv")
                            nc.sync.dma_start(
                                out=dv[:], in_=dinvbc[:, b * P : (b + 1) * P]
                            )
                            dv_tiles[b] = dv
                    agg_t = {
                        b: psagg.tile([P, w], f32, tag="agg", name=f"agg{layer}_{s}_{b}")
                        for b in blocks
                    }
                    # self-loop contribution opens each block's PSUM group
                    for b in blocks:
                        m = min(P, shard - b * P)
                        rl = rl_tiles[b]
                        solo = (s, b) not in ls.has_slots
                        if layer == 1:
                            nc.tensor.matmul(
                                out=agg_t[b][:, :],
                                lhsT=rl[:m, :],
                                rhs=ident_sb[:m, :],
                                start=True,
                                stop=solo,
                            )
                        else:
                            nc.tensor.matmul(
                                out=agg_t[b][:, :],
                                lhsT=ident_sb[:m, :],
                                rhs=rl[:m, :out_dim],
                                start=True,
                                stop=solo,
                            )
                    for k in range(4):
                        for g in seg_gathers.get((s, k), []):
                            if (layer, g["gi"]) in pre_issued:
                                gt = pre_issued.pop((layer, g["gi"]))
                            else:
                                gt = issue_gather(layer, g, it_tiles[g["gi"]])
                            sbig = ind_tiles[g["gi"]]
                            for sl in g_slots.get(g["gi"], []):
                                if layer == 1:
                                    nc.tensor.matmul(
                                        out=agg_t[sl["blk"]][:, :],
                                        lhsT=gt[:, sl["cl"], :],
                                        rhs=sbig[:, sl["sl_local"], :],
                                        start=sl["start"],
                                        stop=sl["stop"],
                                    )
                                else:
                                    nc.tensor.matmul(
                                        out=agg_t[sl["blk"]][:, :],
                                        lhsT=sbig[:, sl["sl_local"], :],
                                        rhs=gt[:, sl["cl"], :out_dim],
                                        start=sl["start"],
                                        stop=sl["stop"],
                                    )
                    # ---- block epilogues
                    for b in blocks:
                        m = min(P, shard - b * P)
                        if layer == 1:
                            t1 = bpool.tile([P, P], bf16, tag="t1")
                            nc.vector.tensor_tensor(
                                out=t1[:],
                                in0=agg_t[b][:, :],
                                in1=dv_tiles[b][:],
                                op=mybir.AluOpType.mult,
                            )
                            o1 = bpool.tile([P, P], bf16, tag="o1")
                            nc.scalar.activation(
                                out=o1[:],
                                in_=t1[:],
                                func=mybir.ActivationFunctionType.Relu,
                                bias=b1_sb[:, :1],
                            )
                            h2p = psh2.tile([P, out_dim], f32, tag="h2p")
                            nc.tensor.matmul(
                                out=h2p[:],
                                lhsT=o1[:],
                                rhs=w2_sb[:],
                                start=True,
                                stop=True,
                            )
                            h2s = bpool.tile([P, P], bf16, tag="h2s")
                            nc.scalar.activation(
                                out=h2s[:m, :out_dim],
                                in_=h2p[:m, :],
                                func=mybir.ActivationFunctionType.Copy,
                                scale=dinvb_sb[:m, b : b + 1],
                            )
                            q = quarter_of(b)
                            r0 = b * P - sch.qrow_start[q]
                            nc.sync.dma_start(
                                out=h2loc_q[q][r0 : r0 + m, 0:out_dim],
                                in_=h2s[:m, :out_dim],
                            )
                        else:
                            t2 = bpool.tile([P, out_dim], f32, tag="t2")
                            nc.scalar.activation(
                                out=t2[:m, :],
                                in_=agg_t[b][:m, :],
                                func=mybir.ActivationFunctionType.Copy,
                                scale=dinvb_sb[:m, b : b + 1],
                            )
                            ob = bpool.tile([P, out_dim], f32, tag="ob")
                            nc.vector.tensor_tensor(
                                out=ob[:m, :],
                                in0=t2[:m, :],
                                in1=b2_sb[:m, :],
                                op=mybir.AluOpType.add,
                            )
                            nc.sync.dma_start(
                                out=out_ext[b * P : b * P + m, :], in_=ob[:m, :]
                            )
                    rl_tiles = next_rl
                    ind_tiles = next_ind
                    if layer == 1:
                        trig = ag2_at_sb.get(s, [])
                        if 3 in trig:
                            # pre-issue layer-2 bank-0..2 gathers so the Q7
                            # queues stay busy while the final AG2 runs
                            for (s2, k2) in l2_pre:
                                for g in seg2.get((s2, k2), []):
                                    it = load_it(2, g)
                                    pre_issued[(2, g["gi"])] = issue_gather(2, g, it)
                        for q in trig:
                            nc.gpsimd.collective_compute(
                                "AllGather",
                                mybir.AluOpType.bypass,
                                ins=[h2loc_q[q][:]],
                                outs=[h2bank[q][:]],
                                replica_groups=[list(range(NCORES))],
                            )
                    it_tiles = next_tiles

            run_layer(1)
            run_layer(2)
            regstack.close()

    nc.compile()
    return nc


# ---------------------------------------------------------------- kernel ---
def _make_in_maps(sch, x, W1, b1v, W2, b2v):
    hid = W1.shape[1]
    out_dim = W2.shape[1]
    shard, nblk = sch.shard, sch.nblk
    bf = ml_dtypes.bfloat16
    in_maps = []
    w1b = W1.astype(bf)
    w2b = W2.astype(bf)
    b1c = b1v.reshape(hid, 1).astype(np.float32).copy()
    b2c = np.broadcast_to(b2v.astype(np.float32), (P, out_dim)).copy()
    iotar = np.tile(np.arange(P, dtype=np.float32), (P, sch.slotmax)).astype(bf)
    ident = np.eye(P, dtype=np.float32).astype(bf)
    xs_all = (x * sch.dinv[:, None]).astype(bf)
    for c in range(NCORES):
        xs = np.ascontiguousarray(xs_all[c * shard : (c + 1) * shard].T)
        dv = sch.dinv[c * shard : (c + 1) * shard]
        full = np.zeros(nblk * P, np.float32)
        full[:shard] = dv
        dvb = np.ascontiguousarray(full.reshape(nblk, P).T)
        dbc = np.broadcast_to(full, (P, nblk * P)).copy()
        in_maps.append(
            {
                "xT": xs,
                "idxs1": sch.L1.idx_stream[c],
                "idxs2": sch.L2.idx_stream[c],
                "dstloc1": sch.L1.dstloc_s[c],
                "dstloc2": sch.L2.dstloc_s[c],
                "dinvb": dvb,
                "W1": w1b,
                "b1": b1c,
                "W2": w2b,
                "b2bc": b2c,
                "iotar": iotar,
                "ident": ident,
                "dinvbc": dbc,
            }
        )
    return in_maps


def _get_compiled(n, e, edge_index, in_dim, hid, out_dim):
    key = ("nc", n, e)
    if key not in _CACHE:
        sch = _preprocess(n, edge_index)
        _CACHE[("sched", n, e)] = sch
        _CACHE[key] = _build(sch, in_dim, hid, out_dim)
    return _CACHE[("sched", n, e)], _CACHE[key]


def kernel(x, edge_index, W1, b1, W2, b2):
    _install_compat()
    from concourse.bass_utils import run_bass_kernel_spmd

    x = np.asarray(x)
    edge_index = np.asarray(edge_index)
    W1 = np.asarray(W1, np.float32)
    b1v = np.asarray(b1, np.float32)
    W2 = np.asarray(W2, np.float32)
    b2v = np.asarray(b2, np.float32)
    n, in_dim = x.shape
    hid = W1.shape[1]
    out_dim = W2.shape[1]

    sch, nc = _get_compiled(n, edge_index.shape[1], edge_index, in_dim, hid, out_dim)
    in_maps = _make_in_maps(sch, x, W1, b1v, W2, b2v)
    import os

    trace = bool(os.environ.get("GCN_TRACE"))
    res = run_bass_kernel_spmd(
        nc, in_maps, core_ids=list(range(NCORES)), trace=trace
    )
    global LAST_EXEC_NS
    LAST_EXEC_NS = res.exec_time_ns
    return np.concatenate([res.results[c]["out"] for c in range(NCORES)], axis=0)


LAST_EXEC_NS = None


# revision 47
# speedup vs baseline: 1.9302x; 1.0993x over previous
"""2-layer GCN (GCNConv x2) on 8 Trainium2 NeuronCores.

Strategy (dst-sharded, edge-partitioned by destination; Q7-descgen-optimized):
- Each core owns N/8 destination nodes and the edges pointing at them
  (plus the GCN self-loops, kept in the edge stream).
- Layer-1 table: h~ = (dinv*x) @ W1 computed per-shard (x host-prescaled by
  dinv), ONE AllGather into hfull; gather banks = 4 int16-addressable row
  slices of hfull (rank-pair layout). A single collective minimizes the
  ~60-95us per-op CC-stream overhead of this stack.
- Layer-2 table: h2~ = dinv * h2 written per-quarter (block-aligned
  quarters), 4 bank-wise AllGathers fired as quarters complete so they
  overlap layer-1's gather phase; bank q = concat over cores of quarter q.
- Since the two layers use different bank layouts, each layer has its own
  edge schedule (idx stream / dstloc / slot table); the dst-side blocking
  is shared.
- Per (super-block of 5 dst blocks, bank): edges packed contiguously in
  block order into 128-slot chunks (straddling block boundaries); one
  dma_gather per segment on queue=bank (4 SWDGE queue contexts, balanced).
  Scatter-add via is_equal-indicator matmuls accumulating in PSUM.
- idx tiles and epilogue dv tiles prefetched one super-block ahead so the
  sync-engine FIFO (epilogue stores) never delays gather dispatch; layer 2's
  first bank-0..2 gathers are pre-issued BEFORE the last AG2 trigger
  (collective triggers block the gpsimd SEQ until completion).
- Layer 1 accumulates transposed (aggT [feat, dst]) so bias+ReLU ride the
  activation engine and out1 feeds h2 = out1 @ W2 directly as lhsT.
"""
import sys
import types

import numpy as np
import ml_dtypes

P = 128
NCORES = 8
GMAX = 32  # max chunks (128 idxs each) per dma_gather
SB_N = 3  # dst blocks per super-block; psagg 2*SB_N banks -> 2 sbs in flight
NQUEUES = 4
XGRP = 8
GBUFS = 16
SBUFS = 10
AG2_LAG = 2  # super-blocks between a quarter finishing and its AG2 trigger

_CACHE = {}


# ---------------------------------------------------------------- compat ---
def _install_compat():
    """Patches for this axon/walrus stack (drain waits, per-inst wait caps,
    NTFF shim). Idempotent."""
    if _CACHE.get("compat"):
        return
    import concourse.tile as tile
    import concourse.mybir as mybir

    _ev = [0]

    def _split_inst_waits(ordered):
        for _bb, insts in ordered.items():
            out = []
            for inst in insts:
                si = getattr(inst, "sync_info", None)
                if si is not None and si.on_wait is not None and len(si.on_wait) > 1:
                    waits = list(si.on_wait)
                    excess, keep = waits[:-1], waits[-1:]
                    si.on_wait.clear()
                    for sw in keep:
                        si.on_wait.append(sw)
                    for i in range(0, len(excess), 2):
                        _ev[0] += 1
                        ev = mybir.InstEventSemaphore(
                            name=f"evsplit-{_ev[0]}", ins=[], outs=[]
                        )
                        ev.engine = inst.engine
                        ev.sync_info = mybir.SyncInfo(
                            on_wait=excess[i : i + 2], on_update=[]
                        )
                        out.append(ev)
                out.append(inst)
            insts[:] = out

    orig_lower = tile.TileContext._lower_ordered_insts

    def patched_lower(self, ordered):
        _split_inst_waits(ordered)
        return orig_lower(self, ordered)

    def patched_drain(self, tick_clock, wait_clock):
        sems_alloc = list(self.sems.allocated().values())
        carrier = self.nc.sync.wait_ge(sems_alloc[0], 0)
        wait_clock.add_sem_waits(
            carrier.ins, tile.ScopedClock({None: tick_clock.global_clock})
        )
        waits = list(carrier.ins.sync_info.on_wait)
        carrier.ins.sync_info.on_wait.clear()
        for sw in waits[:2]:
            carrier.ins.sync_info.on_wait.append(sw)
        for i in range(2, len(waits), 2):
            c = self.nc.sync.wait_ge(sems_alloc[0], 0)
            c.ins.sync_info.on_wait.clear()
            for sw in waits[i : i + 2]:
                c.ins.sync_info.on_wait.append(sw)
        self.nc.sync.drain(fusable=False)
        self.nc.all_engine_barrier()
        popped = self.nc._tile_sem_poison_stack.pop()
        assert popped is self._sem_poison
        self.nc.clear_and_free_semaphores(sems_alloc)
        self.nc.all_engine_barrier()

    tile.TileContext._lower_ordered_insts = patched_lower
    tile.TileContext._drain_and_barrier = patched_drain

    # NTFF profile hook shim (missing antenv.axon_hooks in this image)
    _hook = {}
    mod = types.ModuleType("antenv.axon_hooks")
    mod.set_axon_ntff_profile_hook = lambda h: _hook.update(hook=h)
    mod.get_axon_ntff_profile_hook = lambda: _hook.get("hook")
    sys.modules["antenv.axon_hooks"] = mod
    try:
        import antenv

        antenv.axon_hooks = mod
        from trn_agent_boot.trn_boot import _ntff_profile_via_ctypes

        mod.set_axon_ntff_profile_hook(
            _ntff_profile_via_ctypes("/opt/axon/libaxon_pjrt.so")
        )
    except Exception:
        pass
    _CACHE["compat"] = True


# ---------------------------------------------------------- preprocessing ---
class Schedule:
    pass


class LayerSched:
    pass


def _quarter_bounds(nblk):
    base, rem = nblk // 4, nblk % 4
    sizes = [base + (1 if i < rem else 0) for i in range(4)]
    starts = np.cumsum([0] + sizes)
    return [(int(starts[i]), int(starts[i + 1])) for i in range(4)]


def _make_layer_sched(
    n, nblk, n_sb, s_core, s_sb, s_bank, s_blk, s_bidx, s_dstloc
):
    """Build the per-(super-block, bank) straddle-packed schedule for one
    bank mapping. Inputs are edge arrays sorted by (core, sb, bank, blk)."""
    e = s_core.shape[0]
    key = ((s_core * n_sb + s_sb) * 4 + s_bank) * nblk + s_blk
    cnt = np.bincount(key, minlength=NCORES * n_sb * 4 * nblk).reshape(
        NCORES, n_sb, 4, nblk
    )

    gathers = []
    slots = []
    first_slot_of_block = {}
    last_slot_of_block = {}
    chunk_gid = 0
    slot_gid = 0
    budget_tab = np.zeros((n_sb, 4), np.int64)
    for s in range(n_sb):
        blocks = list(range(s * SB_N, min((s + 1) * SB_N, nblk)))
        for k in range(4):
            percore = cnt[:, s, k, :][:, blocks]
            cum = np.cumsum(percore, axis=1)
            budget = max(int(np.ceil(cum[:, -1].max() / P)), 1)
            budget_tab[s, k] = budget
            lo = np.min(cum - percore, axis=0)
            hi = np.max(cum, axis=0)
            g0 = 0
            while g0 < budget:
                gn = min(GMAX, budget - g0)
                gi = len(gathers)
                gslot0 = slot_gid
                for j in range(g0, g0 + gn):
                    c_lo, c_hi = j * P, (j + 1) * P
                    for bi, b in enumerate(blocks):
                        if hi[bi] > c_lo and lo[bi] < c_hi:
                            slots.append(
                                dict(
                                    g=gi,
                                    cl=j - g0,
                                    blk=b,
                                    sb=s,
                                    bank=k,
                                    sl_local=slot_gid - gslot0,
                                    chunk_gid=chunk_gid + j,
                                )
                            )
                            first_slot_of_block.setdefault((s, b), slot_gid)
                            last_slot_of_block[(s, b)] = slot_gid
                            slot_gid += 1
                gathers.append(
                    dict(
                        gi=gi,
                        sb=s,
                        bank=k,
                        chunk0=chunk_gid + g0,
                        nch=gn,
                        nidx=gn * P,
                        slot0=gslot0,
                        nslots=slot_gid - gslot0,
                        c16=(chunk_gid + g0) * P // 16,
                    )
                )
                g0 += gn
            chunk_gid += budget
    totc = chunk_gid
    nslots = slot_gid
    tot_slots = totc * P
    # the self-loop identity matmul opens each block's PSUM group (start);
    # the last slot closes it (stop)
    for i, sl in enumerate(slots):
        sl["start"] = False
        sl["stop"] = last_slot_of_block[(sl["sb"], sl["blk"])] == i
    has_slots = set(first_slot_of_block.keys())

    seg_key = (s_core * n_sb + s_sb) * 4 + s_bank
    seg_ptr = np.searchsorted(seg_key, np.arange(NCORES * n_sb * 4 + 1))
    seg_chunk0 = {}
    cg = 0
    for s in range(n_sb):
        for k in range(4):
            seg_chunk0[(s, k)] = cg
            cg += int(budget_tab[s, k])

    # one gather per segment required for the per-core dynamic trim
    assert all(budget_tab[s, k] <= GMAX for s in range(n_sb) for k in range(4))
    seg_gi = {}
    for g in gathers:
        seg_gi[(g["sb"], g["bank"])] = g["gi"]

    idx_flat = np.zeros((NCORES, tot_slots), np.int16)
    # per-core per-gather trimmed counts (multiple of 128; trailing full-pad
    # chunks are -1 so the Q7 trims them; num_idxs_reg carries this count so
    # the decode-side ring reservation matches gen_descs)
    cnts = np.zeros((NCORES, len(gathers)), np.int32)
    for c in range(NCORES):
        arr = idx_flat[c]
        for s in range(n_sb):
            for k in range(4):
                p0 = seg_ptr[(c * n_sb + s) * 4 + k]
                p1 = seg_ptr[(c * n_sb + s) * 4 + k + 1]
                base = seg_chunk0[(s, k)] * P
                cnt_c = p1 - p0
                arr[base : base + cnt_c] = s_bidx[p0:p1].astype(np.int16)
                budget = int(budget_tab[s, k])
                part_end = min(max(-(-cnt_c // P), 1) * P, budget * P)
                arr[base + part_end : base + budget * P] = -1
                cnts[c, seg_gi[(s, k)]] = part_end

    dstloc_s = np.full((NCORES, P, nslots), -1.0, np.float32)
    seg_id = (s_core * n_sb + s_sb) * 4 + s_bank
    pos_in_seg = np.arange(e) - seg_ptr[seg_id]
    seg_chunk0_arr = np.zeros(NCORES * n_sb * 4, np.int64)
    for s in range(n_sb):
        for k in range(4):
            for c in range(NCORES):
                seg_chunk0_arr[(c * n_sb + s) * 4 + k] = seg_chunk0[(s, k)]
    chunk_of_edge = seg_chunk0_arr[seg_id] + pos_in_seg // P
    part_of_edge = pos_in_seg % P
    slot_lut = np.full((totc, nblk), -1, np.int64)
    for i, sl in enumerate(slots):
        slot_lut[sl["chunk_gid"], sl["blk"]] = i
    slot_of_edge = slot_lut[chunk_of_edge, s_blk]
    assert (slot_of_edge >= 0).all()
    dstloc_s[s_core, part_of_edge, slot_of_edge] = s_dstloc

    idx_stream = np.ascontiguousarray(
        idx_flat.reshape(NCORES, tot_slots // 16, 16).transpose(0, 2, 1)
    )
    idx_stream = np.tile(idx_stream, (1, 8, 1))

    ls = LayerSched()
    ls.has_slots = has_slots
    ls.cnts = cnts
    ls.gathers = gathers
    ls.slots = slots
    ls.totc, ls.nslots, ls.tot_slots = totc, nslots, tot_slots
    ls.slotmax = max(g["nslots"] for g in gathers)
    ls.gchmax = max(g["nch"] for g in gathers)
    ls.idx_stream = idx_stream
    ls.idx_flat = idx_flat
    ls.dstloc_s = dstloc_s.astype(ml_dtypes.bfloat16)
    return ls


def _preprocess(n, edge_index):
    src0 = np.asarray(edge_index[0], np.int64)
    dst0 = np.asarray(edge_index[1], np.int64)
    shard = n // NCORES
    nblk = (shard + P - 1) // P
    n_sb = (nblk + SB_N - 1) // SB_N
    qb = _quarter_bounds(nblk)
    qrow_start = [b0 * P for b0, b1 in qb]
    qrows = [min(b1 * P, shard) - b0 * P for b0, b1 in qb]
    bank2_n = [NCORES * r for r in qrows]
    assert all(b <= 32767 for b in bank2_n)
    bank1_rows = (n + 3) // 4  # rank-pair banks for layer 1 (slices of hfull)
    assert bank1_rows <= 32767

    deg = np.bincount(dst0, minlength=n).astype(np.float64) + 1.0
    dinv = (1.0 / np.sqrt(deg)).astype(np.float32)

    # self-loops are NOT in the edge stream: both layers fold them in with
    # one identity matmul per block from the per-core hloc/h2loc tensors.
    src = src0
    dst = dst0

    core_d = dst // shard
    dl = dst - core_d * shard
    blk = dl // P
    dstloc = (dl % P).astype(np.int64)
    sb = blk // SB_N

    # layer-1 bank mapping: contiguous row slices of hfull (rank-major)
    bank_a = src // bank1_rows
    bidx_a = src - bank_a * bank1_rows

    # layer-2 bank mapping: quarter-stacked
    core_s = src // shard
    off = src - core_s * shard
    sblk = off // P
    qb_arr = np.zeros(nblk, np.int64)
    for q, (b0, b1) in enumerate(qb):
        qb_arr[b0:b1] = q
    bank_b = qb_arr[sblk]
    bidx_b = core_s * np.array(qrows)[bank_b] + (off - np.array(qrow_start)[bank_b])

    scheds = []
    for bank, bidx in ((bank_a, bidx_a), (bank_b, bidx_b)):
        order = np.lexsort((blk, bank, sb, core_d))
        scheds.append(
            _make_layer_sched(
                n,
                nblk,
                n_sb,
                core_d[order],
                sb[order],
                bank[order],
                blk[order],
                bidx[order],
                dstloc[order],
            )
        )

    sch = Schedule()
    sch.n, sch.shard, sch.nblk, sch.n_sb = n, shard, nblk, n_sb
    sch.e = src.shape[0]
    sch.qb, sch.qrow_start, sch.qrows = qb, qrow_start, qrows
    sch.bank1_rows, sch.bank2_n = bank1_rows, bank2_n
    sch.dinv = dinv
    sch.L1, sch.L2 = scheds
    sch.slotmax = max(sch.L1.slotmax, sch.L2.slotmax)
    return sch


# ----------------------------------------------------------------- build ---
def _build(sch, in_dim, hid, out_dim):
    import concourse.mybir as mybir
    import concourse.tile as tile
    from concourse import bacc

    bf16 = mybir.dt.bfloat16
    f32 = mybir.dt.float32
    shard, nblk, n_sb = sch.shard, sch.nblk, sch.n_sb
    slotmax = sch.slotmax
    gchmax = max(sch.L1.gchmax, sch.L2.gchmax)
    qb = sch.qb
    n = sch.n

    nc = bacc.Bacc(num_swdge_queues=NQUEUES)

    xT = nc.declare_dram_parameter("xT", [in_dim, shard], bf16, isOutput=False)
    idxs1 = nc.declare_dram_parameter(
        "idxs1", [P, sch.L1.tot_slots // 16], mybir.dt.int16, isOutput=False
    )
    idxs2 = nc.declare_dram_parameter(
        "idxs2", [P, sch.L2.tot_slots // 16], mybir.dt.int16, isOutput=False
    )
    dstloc1 = nc.declare_dram_parameter(
        "dstloc1", [P, sch.L1.nslots], bf16, isOutput=False
    )
    dstloc2 = nc.declare_dram_parameter(
        "dstloc2", [P, sch.L2.nslots], bf16, isOutput=False
    )
    iotar_in = nc.declare_dram_parameter("iotar", [P, slotmax * P], bf16, isOutput=False)
    dinvbc = nc.declare_dram_parameter("dinvbc", [P, nblk * P], f32, isOutput=False)
    dinvb = nc.declare_dram_parameter("dinvb", [P, nblk], f32, isOutput=False)
    w1 = nc.declare_dram_parameter("W1", [in_dim, hid], bf16, isOutput=False)
    b1 = nc.declare_dram_parameter("b1", [hid, 1], f32, isOutput=False)
    w2 = nc.declare_dram_parameter("W2", [hid, out_dim], bf16, isOutput=False)
    b2bc = nc.declare_dram_parameter("b2bc", [P, out_dim], f32, isOutput=False)
    ident_in = nc.declare_dram_parameter("ident", [P, P], bf16, isOutput=False)
    cnt1 = nc.declare_dram_parameter(
        "cnt1", [1, len(sch.L1.gathers)], mybir.dt.int32, isOutput=False
    )
    cnt2 = nc.declare_dram_parameter(
        "cnt2", [1, len(sch.L2.gathers)], mybir.dt.int32, isOutput=False
    )
    out_ext = nc.declare_dram_parameter("out", [shard, out_dim], f32, isOutput=True)

    hloc = nc.dram_tensor("hloc", [shard, P], bf16)
    hfull = nc.dram_tensor("hfull", [n, P], bf16, addr_space="Shared")
    h2loc_q = [
        nc.dram_tensor(f"h2loc{q}", [sch.qrows[q], P], bf16) for q in range(4)
    ]
    h2bank = [
        nc.dram_tensor(f"h2bank{q}", [sch.bank2_n[q], P], bf16, addr_space="Shared")
        for q in range(4)
    ]

    kin = in_dim // P

    def quarter_of(b):
        for q, (b0, b1) in enumerate(qb):
            if b0 <= b < b1:
                return q
        raise AssertionError

    def layer_maps(ls):
        seg_gathers = {}
        for g in ls.gathers:
            seg_gathers.setdefault((g["sb"], g["bank"]), []).append(g)
        g_slots = {}
        for sl in ls.slots:
            g_slots.setdefault(sl["g"], []).append(sl)
        return seg_gathers, g_slots

    seg1, gsl1 = layer_maps(sch.L1)
    seg2, gsl2 = layer_maps(sch.L2)

    ag2_at_sb = {}
    for q in range(4):
        sq_end = (qb[q][1] - 1) // SB_N
        key = min(sq_end + AG2_LAG, n_sb - 1) if q < 3 else n_sb - 1
        ag2_at_sb.setdefault(key, []).append(q)

    # layer-2 gathers pre-issued before the final AG2 trigger (the trigger
    # blocks the gpsimd SEQ until the collective completes)
    l2_pre = [(s2, k2) for s2 in range(4) for k2 in range(3)]

    with tile.TileContext(nc) as tc:
        with (
            tc.tile_pool(name="const", bufs=1) as cpool,
            tc.tile_pool(name="xload", bufs=2) as xpool,
            tc.tile_pool(name="hb", bufs=3) as hbpool,
            tc.tile_pool(name="idx", bufs=16) as ipool,
            tc.tile_pool(name="gath", bufs=GBUFS) as gpool,
            tc.tile_pool(name="sind", bufs=SBUFS) as spool,
            tc.tile_pool(name="dvp", bufs=12) as dvpool,
            tc.tile_pool(name="rl", bufs=12) as rpool,
            tc.tile_pool(name="blk", bufs=3) as bpool,
            tc.tile_pool(name="psh", bufs=1, space="PSUM") as psh,
            tc.tile_pool(name="psagg", bufs=2 * SB_N, space="PSUM") as psagg,
            tc.tile_pool(name="psh2", bufs=1, space="PSUM") as psh2,
        ):
            import contextlib

            regstack = contextlib.ExitStack()
            dynreg = regstack.enter_context(nc.gpsimd.register("dyn_nidx"))
            nc.gpsimd.reg_mov(dynreg, 0)

            # ---- constants into SBUF
            w1_t = [
                cpool.tile([P, hid], bf16, tag=f"w1_{k}", name=f"w1t{k}")
                for k in range(kin)
            ]
            for k in range(kin):
                nc.sync.dma_start(out=w1_t[k][:], in_=w1[k * P : (k + 1) * P, :])
            w2_sb = cpool.tile([hid, out_dim], bf16, tag="w2")
            nc.sync.dma_start(out=w2_sb[:], in_=w2[:])
            b1_sb = cpool.tile([hid, 1], f32, tag="b1")
            nc.sync.dma_start(out=b1_sb[:], in_=b1[:])
            b2_sb = cpool.tile([P, out_dim], f32, tag="b2")
            nc.sync.dma_start(out=b2_sb[:], in_=b2bc[:])
            dinvb_sb = cpool.tile([P, nblk], f32, tag="dinvb")
            nc.sync.dma_start(out=dinvb_sb[:], in_=dinvb[:])
            dstloc1_sb = cpool.tile([P, sch.L1.nslots], bf16, tag="dstloc1")
            nc.sync.dma_start(out=dstloc1_sb[:], in_=dstloc1[:])
            dstloc2_sb = cpool.tile([P, sch.L2.nslots], bf16, tag="dstloc2")
            nc.sync.dma_start(out=dstloc2_sb[:], in_=dstloc2[:])
            iotar_sb = cpool.tile([P, slotmax * P], bf16, tag="iotar")
            nc.sync.dma_start(out=iotar_sb[:], in_=iotar_in[:])
            ident_sb = cpool.tile([P, P], bf16, tag="ident")
            nc.sync.dma_start(out=ident_sb[:], in_=ident_in[:])
            cnt1_sb = cpool.tile([1, len(sch.L1.gathers)], mybir.dt.int32, tag="cnt1")
            nc.sync.dma_start(out=cnt1_sb[:], in_=cnt1[:])
            cnt2_sb = cpool.tile([1, len(sch.L2.gathers)], mybir.dt.int32, tag="cnt2")
            nc.sync.dma_start(out=cnt2_sb[:], in_=cnt2[:])

            # zero-init gather buffers: trimmed gathers skip whole chunks, and
            # stale garbage x 0-indicator must not be NaN
            for _ in range(GBUFS):
                gz = gpool.tile([P, gchmax, P], bf16, tag="gt")
                nc.vector.memset(gz[:], 0.0)

            # ---- h~ = (dinv*x) @ W1, shard-local (x pre-scaled by dinv);
            # 4 blocks batched per PSUM bank, double-buffered
            for g0 in range(0, nblk, XGRP):
                g1 = min(g0 + XGRP, nblk)
                c0, c1 = g0 * P, min(g1 * P, shard)
                xt = [
                    xpool.tile([P, XGRP * P], bf16, tag=f"xt{k}", name=f"xt{k}")
                    for k in range(kin)
                ]
                for k in range(kin):
                    nc.sync.dma_start(
                        out=xt[k][:, : c1 - c0], in_=xT[k * P : (k + 1) * P, c0:c1]
                    )
                for s0 in range(g0, g1, 4):
                    s1 = min(s0 + 4, g1)
                    hp = psh.tile([P, 4 * hid], f32, tag="hps")
                    mtot = min(s1 * P, shard) - s0 * P
                    for b in range(s0, s1):
                        m = min(P, shard - b * P)
                        sub = b - s0
                        for k in range(kin):
                            nc.tensor.matmul(
                                out=hp[:m, sub * hid : (sub + 1) * hid],
                                lhsT=xt[k][:, b * P - c0 : b * P - c0 + m],
                                rhs=w1_t[k][:],
                                start=(k == 0),
                                stop=(k == kin - 1),
                            )
                    nsub = s1 - s0
                    hsb = hbpool.tile([P, 4, hid], bf16, tag="hsb")
                    nc.scalar.activation(
                        out=hsb[:, :nsub, :],
                        in_=hp[:, : nsub * hid].rearrange("p (g f) -> p g f", g=nsub),
                        func=mybir.ActivationFunctionType.Copy,
                    )
                    nfull = mtot // P
                    if nfull:
                        nc.sync.dma_start(
                            out=hloc[s0 * P : s0 * P + nfull * P, :].rearrange(
                                "(g p) f -> p g f", p=P
                            ),
                            in_=hsb[:, :nfull, :],
                        )
                    if mtot > nfull * P:
                        mp = mtot - nfull * P
                        nc.sync.dma_start(
                            out=hloc[s0 * P + nfull * P : s0 * P + mtot, :],
                            in_=hsb[:mp, nfull, :],
                        )

            nc.gpsimd.collective_compute(
                "AllGather",
                mybir.AluOpType.bypass,
                ins=[hloc[:]],
                outs=[hfull[:]],
                replica_groups=[list(range(NCORES))],
            )

            def bank_table(layer, k):
                if layer == 1:
                    r0 = k * sch.bank1_rows
                    r1 = min(r0 + sch.bank1_rows, n)
                    return hfull[r0:r1, :]
                return h2bank[k][0 : sch.bank2_n[k], :]

            def issue_gather(layer, g, it):
                nidx, nch = g["nidx"], g["nch"]
                cnt_sb = cnt1_sb if layer == 1 else cnt2_sb
                nc.gpsimd.reg_load(dynreg, cnt_sb[0:1, g["gi"] : g["gi"] + 1])
                gt = gpool.tile([P, gchmax, P], bf16, tag="gt")
                nc.gpsimd.dma_gather(
                    out_ap=gt[:, :nch, :],
                    in_ap=bank_table(layer, g["bank"]),
                    idxs_ap=it[:, : nidx // 16],
                    num_idxs=nidx,
                    num_idxs_reg=dynreg,
                    elem_size=P,
                    single_packet=False,
                    queue_num=g["bank"],
                )
                return gt

            def load_it(layer, g):
                idxs = idxs1 if layer == 1 else idxs2
                it = ipool.tile([P, gchmax * 8], mybir.dt.int16, tag="it")
                nc.sync.dma_start(
                    out=it[:, : g["nidx"] // 16],
                    in_=idxs[:, g["c16"] : g["c16"] + g["nidx"] // 16],
                )
                return it

            pre_issued = {}  # (layer, gi) -> gt tile

            def run_layer(layer):
                seg_gathers = seg1 if layer == 1 else seg2
                g_slots = gsl1 if layer == 1 else gsl2
                ls = sch.L1 if layer == 1 else sch.L2
                dstloc_sb = dstloc1_sb if layer == 1 else dstloc2_sb
                w = P if layer == 1 else out_dim

                def prefetch_sb(s):
                    tiles = {}
                    if s >= n_sb:
                        return tiles
                    for k in range(4):
                        for g in seg_gathers.get((s, k), []):
                            if (layer, g["gi"]) in pre_issued:
                                continue
                            tiles[g["gi"]] = load_it(layer, g)
                    return tiles

                def prefetch_ind(s):
                    tiles = {}
                    if s >= n_sb:
                        return tiles
                    for k in range(4):
                        for g in seg_gathers.get((s, k), []):
                            nsl = g["nslots"]
                            sbig = spool.tile([P, slotmax, P], bf16, tag="sind")
                            nc.vector.tensor_tensor(
                                out=sbig[:, :nsl, :],
                                in0=iotar_sb[:, : nsl * P].rearrange(
                                    "p (k f) -> p k f", k=nsl
                                ),
                                in1=dstloc_sb[
                                    :, g["slot0"] : g["slot0"] + nsl
                                ].to_broadcast([P, nsl, P]),
                                op=mybir.AluOpType.is_equal,
                            )
                            tiles[g["gi"]] = sbig
                    return tiles

                def prefetch_rl(s):
                    tiles = {}
                    if s >= n_sb:
                        return tiles
                    for b in range(s * SB_N, min((s + 1) * SB_N, nblk)):
                        m = min(P, shard - b * P)
                        rl = rpool.tile([P, P], bf16, tag="rl")
                        if layer == 1:
                            nc.sync.dma_start(
                                out=rl[:m, :], in_=hloc[b * P : b * P + m, :]
                            )
                        else:
                            q = quarter_of(b)
                            r0 = b * P - sch.qrow_start[q]
                            nc.sync.dma_start(
                                out=rl[:m, :], in_=h2loc_q[q][r0 : r0 + m, :]
                            )
                        tiles[b] = rl
                    return tiles

                it_tiles = prefetch_sb(0)
                rl_tiles = prefetch_rl(0)
                ind_tiles = prefetch_ind(0)
                for s in range(n_sb):
                    blocks = list(range(s * SB_N, min((s + 1) * SB_N, nblk)))
                    next_tiles = prefetch_sb(s + 1)
                    next_ind = prefetch_ind(s + 1)
                    next_rl = prefetch_rl(s + 1)
                    dv_tiles = {}
                    if layer == 1:
                        for b in blocks:
                            dv = dvpool.tile([P, P], f32, tag="dv")
                            nc.sync.dma_start(
                                out=dv[:], in_=dinvbc[:, b * P : (b + 1) * P]
                            )
                            dv_tiles[b] = dv
                    agg_t = {
                        b: psagg.tile([P, w], f32, tag="agg", name=f"agg{layer}_{s}_{b}")
                        for b in blocks
                    }
                    # self-loop contribution opens each block's PSUM group
                    for b in blocks:
                        m = min(P, shard - b * P)
                        rl = rl_tiles[b]
                        solo = (s, b) not in ls.has_slots
                        if layer == 1:
                            nc.tensor.matmul(
                                out=agg_t[b][:, :],
                                lhsT=rl[:m, :],
                                rhs=ident_sb[:m, :],
                                start=True,
                                stop=solo,
                            )
                        else:
                            nc.tensor.matmul(
                                out=agg_t[b][:, :],
                                lhsT=ident_sb[:m, :],
                                rhs=rl[:m, :out_dim],
                                start=True,
                                stop=solo,
                            )
                    for k in range(4):
                        for g in seg_gathers.get((s, k), []):
                            if (layer, g["gi"]) in pre_issued:
                                gt = pre_issued.pop((layer, g["gi"]))
                            else:
                                gt = issue_gather(layer, g, it_tiles[g["gi"]])
                            sbig = ind_tiles[g["gi"]]
                            for sl in g_slots.get(g["gi"], []):
                                if layer == 1:
                                    nc.tensor.matmul(
                                        out=agg_t[sl["blk"]][:, :],
                                        lhsT=gt[:, sl["cl"], :],
                                        rhs=sbig[:, sl["sl_local"], :],
                                        start=sl["start"],
                                        stop=sl["stop"],
                                    )
                                else:
                                    nc.tensor.matmul(
                                        out=agg_t[sl["blk"]][:, :],
                                        lhsT=sbig[:, sl["sl_local"], :],
                                        rhs=gt[:, sl["cl"], :out_dim],
                                        start=sl["start"],
                                        stop=sl["stop"],
                                    )
                    # ---- block epilogues
                    for b in blocks:
                        m = min(P, shard - b * P)
                        if layer == 1:
                            t1 = bpool.tile([P, P], bf16, tag="t1")
                            nc.vector.tensor_tensor(
                                out=t1[:],
                                in0=agg_t[b][:, :],
                                in1=dv_tiles[b][:],
                                op=mybir.AluOpType.mult,
                            )
                            o1 = bpool.tile([P, P], bf16, tag="o1")
                            nc.scalar.activation(
                                out=o1[:],
                                in_=t1[:],
                                func=mybir.ActivationFunctionType.Relu,
                                bias=b1_sb[:, :1],
                            )
                            h2p = psh2.tile([P, out_dim], f32, tag="h2p")
                            nc.tensor.matmul(
                                out=h2p[:],
                                lhsT=o1[:],
                                rhs=w2_sb[:],
                                start=True,
                                stop=True,
                            )
                            h2s = bpool.tile([P, P], bf16, tag="h2s")
                            nc.scalar.activation(
                                out=h2s[:m, :out_dim],
                                in_=h2p[:m, :],
                                func=mybir.ActivationFunctionType.Copy,
                                scale=dinvb_sb[:m, b : b + 1],
                            )
                            q = quarter_of(b)
                            r0 = b * P - sch.qrow_start[q]
                            nc.sync.dma_start(
                                out=h2loc_q[q][r0 : r0 + m, 0:out_dim],
                                in_=h2s[:m, :out_dim],
                            )
                        else:
                            t2 = bpool.tile([P, out_dim], f32, tag="t2")
                            nc.scalar.activation(
                                out=t2[:m, :],
                                in_=agg_t[b][:m, :],
                                func=mybir.ActivationFunctionType.Copy,
                                scale=dinvb_sb[:m, b : b + 1],
                            )
                            ob = bpool.tile([P, out_dim], f32, tag="ob")
                            nc.vector.tensor_tensor(
                                out=ob[:m, :],
                                in0=t2[:m, :],
                                in1=b2_sb[:m, :],
                                op=mybir.AluOpType.add,
                            )
                            nc.sync.dma_start(
                                out=out_ext[b * P : b * P + m, :], in_=ob[:m, :]
                            )
                    rl_tiles = next_rl
                    ind_tiles = next_ind
                    if layer == 1:
                        trig = ag2_at_sb.get(s, [])
                        if 3 in trig:
                            # pre-issue layer-2 bank-0..2 gathers so the Q7
                            # queues stay busy while the final AG2 runs
                            for (s2, k2) in l2_pre:
                                for g in seg2.get((s2, k2), []):
                                    it = load_it(2, g)
                                    pre_issued[(2, g["gi"])] = issue_gather(2, g, it)
                        for q in trig:
                            nc.gpsimd.collective_compute(
                                "AllGather",
                                mybir.AluOpType.bypass,
                                ins=[h2loc_q[q][:]],
                                outs=[h2bank[q][:]],
                                replica_groups=[list(range(NCORES))],
                            )
                    it_tiles = next_tiles

            run_layer(1)
            run_layer(2)
            regstack.close()

    nc.compile()
    return nc


# ---------------------------------------------------------------- kernel ---
def _make_in_maps(sch, x, W1, b1v, W2, b2v):
    hid = W1.shape[1]
    out_dim = W2.shape[1]
    shard, nblk = sch.shard, sch.nblk
    bf = ml_dtypes.bfloat16
    in_maps = []
    w1b = W1.astype(bf)
    w2b = W2.astype(bf)
    b1c = b1v.reshape(hid, 1).astype(np.float32).copy()
    b2c = np.broadcast_to(b2v.astype(np.float32), (P, out_dim)).copy()
    iotar = np.tile(np.arange(P, dtype=np.float32), (P, sch.slotmax)).astype(bf)
    ident = np.eye(P, dtype=np.float32).astype(bf)
    xs_all = (x * sch.dinv[:, None]).astype(bf)
    for c in range(NCORES):
        xs = np.ascontiguousarray(xs_all[c * shard : (c + 1) * shard].T)
        dv = sch.dinv[c * shard : (c + 1) * shard]
        full = np.zeros(nblk * P, np.float32)
        full[:shard] = dv
        dvb = np.ascontiguousarray(full.reshape(nblk, P).T)
        dbc = np.broadcast_to(full, (P, nblk * P)).copy()
        in_maps.append(
            {
                "xT": xs,
                "idxs1": sch.L1.idx_stream[c],
                "idxs2": sch.L2.idx_stream[c],
                "dstloc1": sch.L1.dstloc_s[c],
                "dstloc2": sch.L2.dstloc_s[c],
                "dinvb": dvb,
                "W1": w1b,
                "b1": b1c,
                "W2": w2b,
                "b2bc": b2c,
                "iotar": iotar,
                "ident": ident,
                "cnt1": sch.L1.cnts[c : c + 1],
                "cnt2": sch.L2.cnts[c : c + 1],
                "dinvbc": dbc,
            }
        )
    return in_maps


def _get_compiled(n, e, edge_index, in_dim, hid, out_dim):
    key = ("nc", n, e)
    if key not in _CACHE:
        sch = _preprocess(n, edge_index)
        _CACHE[("sched", n, e)] = sch
        _CACHE[key] = _build(sch, in_dim, hid, out_dim)
    return _CACHE[("sched", n, e)], _CACHE[key]


def kernel(x, edge_index, W1, b1, W2, b2):
    _install_compat()
    from concourse.bass_utils import run_bass_kernel_spmd

    x = np.asarray(x)
    edge_index = np.asarray(edge_index)
    W1 = np.asarray(W1, np.float32)
    b1v = np.asarray(b1, np.float32)
    W2 = np.asarray(W2, np.float32)
    b2v = np.asarray(b2, np.float32)
    n, in_dim = x.shape
    hid = W1.shape[1]
    out_dim = W2.shape[1]

    sch, nc = _get_compiled(n, edge_index.shape[1], edge_index, in_dim, hid, out_dim)
    in_maps = _make_in_maps(sch, x, W1, b1v, W2, b2v)
    import os

    trace = bool(os.environ.get("GCN_TRACE"))
    res = run_bass_kernel_spmd(
        nc, in_maps, core_ids=list(range(NCORES)), trace=trace
    )
    global LAST_EXEC_NS
    LAST_EXEC_NS = res.exec_time_ns
    return np.concatenate([res.results[c]["out"] for c in range(NCORES)], axis=0)


LAST_EXEC_NS = None
